# revision 18
# baseline (speedup 1.0000x reference)
"""Trainium2 Bass kernel for nn_ChessGraphPooling (segment_reduce).

Data-parallel over whole graphs: 4096 boards x 64 nodes sharded across 8
NeuronCores (512 graphs / 32768 nodes per core); small weights replicated.

v3 design (bf16 pipeline, engine-balanced):
  - x converted to bf16 on host; per-core DMA traffic halves.
  - node-layout x tiles feed pooling matmuls as stationary operands and are
    transposed into channel-major T-layout [c, nodes] by the DMA engines'
    XBAR transpose (zero PE/DVE cost).
  - scorer linears run on the PE in T-layout; hidden activations (LeakyRelu)
    on the scalar engine; tiny second layers are [10,512] matmuls.
  - segment softmax skips the max-subtract pass (exact: softmax is
    shift-invariant and scores are O(1), exp cannot overflow); scores are
    stacked per-mega (8 supertiles, 80 rows) so every softmax op is one
    free-size-bound pass.
  - all rsqrt = Exp(-0.5*Ln(x)) so the ACT engine keeps one activation
    table (Prelu/Exp/Ln/Square live in natural_log_exp_and_others) and never
    pays the 1.3us table reload.
  - strategic branch LN runs in node layout via bn_stats; normalize+relu is
    a single fused tensor_scalar (sub,mult / mult,max) when gamma==1/beta==0
    (detected on host, the general path adds the row ops back).
  - pooling writes T-layout directly: stationary = x/sf node-layout chunks,
    moving = per-graph weight columns; pooled results land in PSUM already
    channel-major, so no pool transposes and no re-staging.
  - post stage (per-graph MLPs) reuses the same tricks: bf16 matmuls, DMA
    transposes, Pool-engine PSUM drains.
"""

import os
import sys

sys.path.insert(0, "/opt/trn_rl_repo")

from contextlib import ExitStack

import numpy as np
import ml_dtypes

import concourse.bass as bass
import concourse.bacc as bacc
import concourse.tile as tile
import concourse.mybir as mybir
from concourse.bass_utils import run_bass_kernel_spmd

F32 = mybir.dt.float32
BF16 = mybir.dt.bfloat16
FP8 = mybir.dt.float8e4
I32 = mybir.dt.int32
AF = mybir.ActivationFunctionType
OP = mybir.AluOpType
AX = mybir.AxisListType
DRMODE = mybir.MatmulPerfMode.DoubleRow

C = 256
H = 8
NODES = 64
NEG = 0.2
N_CORES = 8
ST = 512          # nodes per supertile
CHUNKS = 4        # 128-node chunks per supertile
MEGA = 8          # supertiles per megatile (80 score rows)
FULL_N_MEGA = 8   # megatiles per core at full size

# matmul input mode: "bf16" or "fp8" (DoubleRow) for the scorer/strat linears
MM_MODE = os.environ.get("K_MM_MODE", "fp8")
# CoreSim does not implement Prelu; sim_safe swaps it for an Abs-based
# decomposition (0.6*v + 0.4*|v|) with identical numerics
SIM_SAFE = bool(os.environ.get("K_SIM_SAFE"))


def build_nc(n_mega=FULL_N_MEGA, flags=None, mm_mode=MM_MODE,
             sim_safe=None, repeat=1):
    flags = dict(flags or {})
    flags["_sim_safe"] = SIM_SAFE if sim_safe is None else sim_safe
    nodes_pc = n_mega * MEGA * ST
    graphs_pc = nodes_pc // NODES
    assert graphs_pc % 128 == 0

    nc = bacc.Bacc("TRN2", num_devices=N_CORES)

    dt = {}

    def din(name, shape, dtype=F32):
        dt[name] = nc.dram_tensor(name, shape, dtype, kind="ExternalInput")

    din("x", [nodes_pc, C], BF16)
    din("mb", [nodes_pc], BF16)       # piece mask (node_types > 0)
    din("nb", [nodes_pc], BF16)       # 1 - piece mask
    din("w1", [128, 2, 512], BF16)    # att W1, [p, k, h*64+d]
    din("b1a", [128, 4])              # att b1 cols per m-chunk
    din("pew", [128, 2, 256], BF16)   # piece|empty W1
    din("b1p", [128, 2])
    din("w2c", [128, 4, 16], BF16)    # att w2 -> score rows 0..7
    din("pw2c", [128, 2, 16], BF16)   # piece/empty w2 -> rows 8,9
    din("b2col", [80, 1])
    din("spw", [128, 2, 256], BF16)
    din("spb", [1, 256], BF16)
    din("spg", [1, 256])
    din("spbt", [1, 256])
    din("sa", [128, 1])
    din("sa2", [128, 1])
    din("cw", [128, 16, 256], BF16)
    din("cb", [1, 256], BF16)
    din("cg", [1, 256])
    din("cbt", [1, 256])
    din("hw", [128, 4, 256], BF16)
    din("hb", [1, 256], BF16)
    din("hg", [1, 256])
    din("hbt", [1, 256])
    din("p1w", [128, 6, 512], BF16)
    din("p1b", [1, 512], BF16)
    din("p1g", [1, 512])
    din("p1bt", [1, 512])
    din("p2w", [128, 4, 256], BF16)
    din("p2b", [1, 256], BF16)
    if mm_mode == "fp8":
        din("w1q", [128, 2, 512], FP8)
        din("pewq", [128, 2, 256], FP8)
        din("spwq", [128, 2, 256], FP8)
        din("w2cq", [128, 2, 2, 16], FP8)   # [p, i, mpair, col]
        din("pw2cq", [128, 2, 16], FP8)     # pe pair: [p, i(m), col]
    out_d = nc.dram_tensor("out", [graphs_pc, C], F32, kind="ExternalOutput")

    with tile.TileContext(nc) as tc:
        for _rep in range(repeat):
            _build_body(nc, tc, n_mega, graphs_pc, dt, out_d, flags, mm_mode)
    nc.compile()
    return nc


def _bcast(nc, dst, src_d):
    nc.gpsimd.dma_start(
        out=dst, in_=src_d.ap().partition_broadcast(dst.shape[0])
    )


def _build_body(nc, tc, n_mega, graphs_pc, dt, out_d, flags, mm_mode):
    gchunks = graphs_pc // 128
    fp8 = mm_mode == "fp8"

    nc.scalar.add_instruction(mybir.InstLoadActFuncSet(
        name=nc.get_next_instruction_name(), ins=[], outs=[],
        act_func_set_id=6))  # natural_log_exp_and_others: Prelu+Exp+Ln

    with ExitStack() as top:
        consts = top.enter_context(tc.tile_pool(name="consts", bufs=1))
        persist = top.enter_context(tc.tile_pool(name="persist", bufs=1))

        def cload(name, shape, dtype=F32, src=None):
            t = consts.tile(shape, dtype, tag=name, name=name)
            nc.sync.dma_start(out=t, in_=dt[src or name].ap())
            return t

        w1 = cload("w1", [128, 2, 512], BF16)
        b1a = cload("b1a", [128, 4])
        pew = cload("pew", [128, 2, 256], BF16)
        b1p = cload("b1p", [128, 2])
        w2c = cload("w2c", [128, 4, 16], BF16)
        pw2c = cload("pw2c", [128, 2, 16], BF16)
        b2col = cload("b2col", [80, 1])
        spw = cload("spw", [128, 2, 256], BF16)
        sa = cload("sa", [128, 1])
        sa2 = cload("sa2", [128, 1])
        if fp8:
            w1q = cload("w1q", [128, 2, 512], FP8)
            pewq = cload("pewq", [128, 2, 256], FP8)
            spwq = cload("spwq", [128, 2, 256], FP8)
            w2cq = cload("w2cq", [128, 2, 2, 16], FP8)
            pw2cq = cload("pw2cq", [128, 2, 16], FP8)

        ones1 = consts.tile([1, 128], BF16, tag="ones1")
        nc.vector.memset(ones1, 1.0)
        spbrow = None
        if not flags.get("spb0", False):
            spbrow = cload("spbrow", [1, 256], BF16, src="spb")
        spgB = spbtB = None
        if not flags.get("spg1", False):
            spgB = consts.tile([128, 256], F32, tag="spgB")
            _bcast(nc, spgB, dt["spg"])
        if not flags.get("spbt0", False):
            spbtB = consts.tile([128, 256], F32, tag="spbtB")
            _bcast(nc, spbtB, dt["spbt"])

        poolcol = consts.tile([128, 2], BF16, tag="poolcol")
        nc.vector.memset(poolcol, 0.0)
        nc.vector.memset(poolcol[0:64, 0:1], 1.0 / NODES)
        nc.vector.memset(poolcol[64:128, 1:2], 1.0 / NODES)
        blockmask = consts.tile([128, 2, 10], BF16, tag="blockmask")
        nc.vector.memset(blockmask, 0.0)
        nc.vector.memset(blockmask[0:64, 0, :], 1.0)
        nc.vector.memset(blockmask[64:128, 1, :], 1.0)
        # double-buffered mask stacks; att rows stay 1.0 forever
        maskS = []
        for i in range(2):
            mt = consts.tile([80, 512], BF16, tag=f"maskS{i}", name=f"maskS{i}")
            nc.vector.memset(mt, 1.0)
            maskS.append(mt)

        staged_x = persist.tile(
            [128, 2, graphs_pc * 10], BF16, tag="staged_x")
        staged_sf = persist.tile([128, 2, graphs_pc], BF16, tag="staged_sf")

        with ExitStack() as main:
            xpool = main.enter_context(tc.tile_pool(name="xpool", bufs=9))
            xTp = main.enter_context(tc.tile_pool(name="xTp", bufs=3))
            actp = main.enter_context(tc.tile_pool(name="actp", bufs=4))
            sfp = main.enter_context(tc.tile_pool(name="sfp", bufs=3))
            stp = main.enter_context(tc.tile_pool(name="stp", bufs=3))
            wcp = main.enter_context(tc.tile_pool(name="wcp", bufs=3))
            scr = main.enter_context(tc.tile_pool(name="scr", bufs=4))
            megap = main.enter_context(tc.tile_pool(name="megap", bufs=2))

            ps_mm = main.enter_context(
                tc.tile_pool(name="ps_mm", bufs=3, space="PSUM"))
            ps_sc = main.enter_context(
                tc.tile_pool(name="ps_sc", bufs=2, space="PSUM"))
            ps_pool = main.enter_context(
                tc.tile_pool(name="ps_pool", bufs=2, space="PSUM"))
            ps_sfp = main.enter_context(
                tc.tile_pool(name="ps_sfp", bufs=1, space="PSUM"))

            for mg in range(n_mega):
                _mega_body(
                    nc, tc, mg, dt, staged_x, staged_sf,
                    w1q if fp8 else w1, b1a, pewq if fp8 else pew, b1p,
                    w2cq if fp8 else w2c, pw2cq if fp8 else pw2c, b2col,
                    spwq if fp8 else spw, spbrow, spgB, spbtB, sa, sa2,
                    ones1, poolcol, blockmask, maskS[mg % 2],
                    xpool, xTp, actp, sfp, stp, wcp, scr, megap,
                    ps_mm, ps_sc, ps_pool, ps_sfp, flags, fp8,
                )

        with ExitStack() as post:
            posw = post.enter_context(tc.tile_pool(name="posw", bufs=1))
            pos = post.enter_context(tc.tile_pool(name="pos", bufs=4))
            posT = post.enter_context(tc.tile_pool(name="posT", bufs=1))
            ps_po = post.enter_context(
                tc.tile_pool(name="ps_po", bufs=2, space="PSUM"))
            ps_pz = post.enter_context(
                tc.tile_pool(name="ps_pz", bufs=2, space="PSUM"))
            _post_body(
                nc, tc, graphs_pc, gchunks, dt, staged_x, staged_sf,
                ones1, posw, pos, posT, ps_po, ps_pz, out_d, flags,
            )


def _prelu(nc, scr, out, ph, bias_col, sim_safe):
    """out = LeakyRelu(ph + bias, NEG)."""
    if not sim_safe:
        nc.scalar.activation(
            out=out, in_=ph, func=AF.Prelu,
            bias=bias_col, scale=1.0, alpha=NEG)
        return
    shape = [ph.shape[0], ph.shape[-1]]
    absv = scr.tile(shape, F32, tag="pabs", name="pabs")
    nc.scalar.activation(
        out=absv, in_=ph, func=AF.Abs, bias=bias_col, scale=1.0)
    t = scr.tile(shape, F32, tag="pt", name="pt")
    nc.vector.tensor_scalar(
        out=t, in0=ph, scalar1=bias_col, scalar2=(1.0 + NEG) / 2.0,
        op0=OP.add, op1=OP.mult)
    nc.vector.scalar_tensor_tensor(
        out=out, in0=absv, scalar=(1.0 - NEG) / 2.0,
        in1=t, op0=OP.mult, op1=OP.add)


def _mega_body(
    nc, tc, mg, dt, staged_x, staged_sf,
    w1, b1a, pew, b1p, w2c, pw2c, b2col, spw, spbrow, spgB, spbtB, sa, sa2,
    ones1, poolcol, blockmask, maskS,
    xpool, xTp, actp, sfp, stp, wcp, scr, megap,
    ps_mm, ps_sc, ps_pool, ps_sfp, flags, fp8,
):
    sim_safe = flags.get("_sim_safe", False)
    spb0 = flags.get("spb0", False)
    spg1 = flags.get("spg1", False)
    spbt0 = flags.get("spbt0", False)
    b20 = flags.get("b20", False)
    mmdt = FP8 if fp8 else BF16

    # per-mega pe-score masks: blocked rows (att 0:64, piece 64:72, empty 72:80)
    nc.sync.dma_start(
        out=maskS[64:72, :],
        in_=dt["mb"].ap()[mg * MEGA * ST:(mg + 1) * MEGA * ST]
        .rearrange("(s n) -> s n", s=8),
    )
    nc.sync.dma_start(
        out=maskS[72:80, :],
        in_=dt["nb"].ap()[mg * MEGA * ST:(mg + 1) * MEGA * ST]
        .rearrange("(s n) -> s n", s=8),
    )

    sstack = megap.tile([80, 512], BF16, tag="sstack")
    stbuf = megap.tile([16, 8, 512], BF16, tag="stbuf")
    xs = []
    xsb2 = None
    for s8 in range(MEGA):
        s = mg * MEGA + s8
        if s8 % 2 == 0:
            xsb2 = xpool.tile([128, 8, 256], BF16, tag="xsb")
            nc.sync.dma_start(
                out=xsb2,
                in_=dt["x"].ap()[s * ST:(s + 2) * ST, :]
                .rearrange("(c p) m -> p c m", p=128),
            )
        xsb = xsb2[:, (s8 % 2) * 4:(s8 % 2) * 4 + 4, :]
        xs.append(xsb)

        # one XBAR transpose per supertile over the flat [128,1024] view:
        # xTf[p, 2c+k, n] = x[node c*128+n, k*128+p]
        xTf = xTp.tile([128, 8, 128], BF16, tag="xT")
        nc.sync.dma_start_transpose(
            out=xTf, in_=xsb.rearrange("p c m -> p (c m)"))
        if fp8:
            # shuffle-convert to standard DR layout [p, k, c*128+n]
            xTq = xTp.tile([128, 2, 512], FP8, tag="xTq", name="xTq")
            eng = nc.gpsimd
            eng.tensor_copy(
                out=xTq.rearrange("p k (c n) -> p k c n", n=128),
                in_=xTf.rearrange("p (c k) n -> p k c n", k=2))
            xmv = xTq
        else:
            xmv = xTf.rearrange("p (c k) n -> p k c n", k=2)

        scp = ps_sc.tile([16, 512], F32, tag="scp")
        # attention scorer
        hLt = None
        for m in range(4):
            ph = ps_mm.tile([128, 512], F32, tag="ph")
            if fp8:
                nc.tensor.matmul(
                    ph, w1[:, :, m * 128:(m + 1) * 128], xmv,
                    start=True, stop=True, perf_mode=DRMODE)
            else:
                nc.tensor.matmul(
                    ph, w1[:, 0, m * 128:(m + 1) * 128], xmv[:, 0],
                    start=True, stop=False)
                nc.tensor.matmul(
                    ph, w1[:, 1, m * 128:(m + 1) * 128], xmv[:, 1],
                    start=False, stop=True)
            if m % 2 == 0:
                hLt = actp.tile([128, 2, 512], mmdt, tag="hL")
            _prelu(nc, scr, hLt[:, m % 2, :], ph, b1a[:, m:m + 1], sim_safe)
            if fp8:
                if m % 2 == 1:
                    nc.tensor.matmul(
                        scp[0:10, :], w2c[:, :, m // 2, 0:10], hLt,
                        start=(m == 1), stop=False, perf_mode=DRMODE)
            else:
                nc.tensor.matmul(
                    scp[0:10, :], w2c[:, m, 0:10], hLt[:, m % 2, :],
                    start=(m == 0), stop=False)

        # piece/empty scorer
        peLt = actp.tile([128, 2, 512], mmdt, tag="peL", name="peL")
        for m in range(2):
            pp = ps_mm.tile([128, 512], F32, tag="ph", name="pp")
            if fp8:
                nc.tensor.matmul(
                    pp, pew[:, :, m * 128:(m + 1) * 128], xmv,
                    start=True, stop=True, perf_mode=DRMODE)
            else:
                nc.tensor.matmul(
                    pp, pew[:, 0, m * 128:(m + 1) * 128], xmv[:, 0],
                    start=True, stop=False)
                nc.tensor.matmul(
                    pp, pew[:, 1, m * 128:(m + 1) * 128], xmv[:, 1],
                    start=False, stop=True)
            _prelu(nc, scr, peLt[:, m, :], pp, b1p[:, m:m + 1], sim_safe)
        if fp8:
            nc.tensor.matmul(
                scp[0:10, :], pw2c[:, :, 0:10], peLt,
                start=False, stop=True, perf_mode=DRMODE)
        else:
            for m in range(2):
                nc.tensor.matmul(
                    scp[0:10, :], pw2c[:, m, 0:10], peLt[:, m, :],
                    start=False, stop=(m == 1))

        # drain scores into the SBUF stage buffer (no DMA)
        nc.vector.tensor_copy(out=stbuf[0:10, s8, :], in_=scp[0:10, :])

        # strategic branch: z' = x @ (spW - rowmean(spW)) is exactly
        # centered, so LN needs only the variance; rho is computed
        # per-supertile and the PSUM drain fuses scale+relu in one op.
        mv4 = scr.tile([128, 4, 2], F32, tag="mv4", name="mv4")
        pzs = []
        for cp in range(2):
            pz = ps_mm.tile([128, 512], F32, tag="ph", name="pz")
            pzs.append(pz)
            for half in range(2):
                sec = cp * 2 + half
                sl = pz[:, half * 256:(half + 1) * 256]
                last = spb0
                if fp8:
                    nc.tensor.matmul(
                        sl, xmv[:, :, sec * 128:(sec + 1) * 128], spw,
                        start=True, stop=last, perf_mode=DRMODE)
                else:
                    nc.tensor.matmul(
                        sl, xmv[:, 0, sec], spw[:, 0, :],
                        start=True, stop=False)
                    nc.tensor.matmul(
                        sl, xmv[:, 1, sec], spw[:, 1, :],
                        start=False, stop=last)
                if not spb0:
                    nc.tensor.matmul(sl, ones1, spbrow, start=False, stop=True)
            for half in range(2):
                sec = cp * 2 + half
                sl = pz[:, half * 256:(half + 1) * 256]
                st6 = scr.tile([128, 6], F32, tag="st6")
                nc.vector.bn_stats(out=st6, in_=sl)
                nc.vector.bn_aggr(out=mv4[:, sec, :], in_=st6)
        rho4 = scr.tile([128, 4], F32, tag="rho4", name="rho4")
        nc.vector.tensor_scalar(
            out=rho4, in0=mv4[:, :, 1], scalar1=sa2, scalar2=1e-5,
            op0=OP.mult, op1=OP.add)
        nc.scalar.activation(out=rho4, in_=rho4, func=AF.Ln)
        nc.scalar.activation(out=rho4, in_=rho4, func=AF.Exp, scale=-0.5)
        nc.vector.tensor_scalar(
            out=rho4, in0=rho4, scalar1=sa, scalar2=None, op0=OP.mult)
        sff = sfp.tile([128, 4, 256], BF16, tag="sf")
        for cp in range(2):
            pz = pzs[cp]
            for half in range(2):
                sec = cp * 2 + half
                sl = pz[:, half * 256:(half + 1) * 256]
                rcol = rho4[:, sec:sec + 1]
                if spg1 and spbt0:
                    if sec != 0:
                        nc.vector.tensor_scalar(
                            out=sff[:, sec, :], in0=sl, scalar1=rcol,
                            scalar2=0.0, op0=OP.mult, op1=OP.max)
                    else:
                        nc.scalar.activation(
                            out=sff[:, sec, :], in_=sl, func=AF.Relu,
                            scale=rcol, bias=0.0)
                else:
                    tg = scr.tile([128, 256], F32, tag="tg", name="tg")
                    nc.vector.tensor_scalar(
                        out=tg, in0=sl, scalar1=rcol, scalar2=None,
                        op0=OP.mult)
                    if spgB is not None:
                        nc.vector.tensor_tensor(
                            out=tg, in0=tg, in1=spgB, op=OP.mult)
                    if spbtB is not None:
                        nc.vector.tensor_tensor(
                            out=tg, in0=tg, in1=spbtB, op=OP.add)
                    nc.vector.tensor_scalar(
                        out=sff[:, sec, :], in0=tg, scalar1=0.0,
                        scalar2=None, op0=OP.max)
        # sf mean-pool straight to T-layout and stage it (phase A)
        psf = ps_sfp.tile([128, 2, 8], F32, tag="psf")
        for sec in range(CHUNKS):
            for k in range(2):
                nc.tensor.matmul(
                    psf[:, k, sec * 2:sec * 2 + 2],
                    sff[:, sec, k * 128:(k + 1) * 128], poolcol,
                    start=True, stop=True)
        for k in range(2):
            nc.vector.tensor_copy(
                out=staged_sf[:, k, s * 8:(s + 1) * 8], in_=psf[:, k, :])

    # assemble the mega score stack: 3 DMAs
    nc.sync.dma_start(out=sstack[0:64, :], in_=stbuf[0:8])
    nc.sync.dma_start(out=sstack[64:72, :], in_=stbuf[8:9])
    nc.sync.dma_start(out=sstack[72:80, :], in_=stbuf[9:10])

    # ---- phase B: batched segment softmax (no max-subtract; exact) ----
    if b20:
        nc.gpsimd.tensor_tensor(
            out=sstack, in0=sstack, in1=maskS, op=OP.mult)
    else:
        nc.gpsimd.scalar_tensor_tensor(
            out=sstack, in0=sstack, scalar=b2col,
            in1=maskS, op0=OP.add, op1=OP.mult)
    estack = megap.tile([80, 512], BF16, tag="estack")
    nc.scalar.activation(out=estack, in_=sstack, func=AF.Exp)
    dsum = megap.tile([80, 8], F32, tag="dsum")
    nc.vector.tensor_reduce(
        out=dsum, in_=estack.rearrange("p (g n) -> p g n", n=NODES),
        axis=AX.X, op=OP.add)
    nc.vector.tensor_scalar(
        out=dsum, in0=dsum, scalar1=1e-16, scalar2=None, op0=OP.add)
    dre = megap.tile([80, 8], BF16, tag="dre")
    with nc.allow_low_precision(reason="softmax denom reciprocal, bf16 ok"):
        nc.vector.reciprocal(out=dre, in_=dsum)
    wT = megap.tile([80, 512], BF16, tag="wT")
    nc.gpsimd.tensor_tensor(
        out=wT.rearrange("p (g n) -> p g n", n=NODES),
        in0=estack.rearrange("p (g n) -> p g n", n=NODES),
        in1=dre.unsqueeze(2).broadcast_to([80, 8, NODES]),
        op=OP.mult)
    # wtt[p, c, r] = wT[r, c*128+p]
    wtt = megap.tile([128, 4, 80], BF16, tag="wtt")
    nc.sync.dma_start_transpose(out=wtt, in_=wT)

    # ---- phase C: attention/piece/empty pooling ----
    for s8 in range(MEGA):
        s = mg * MEGA + s8
        xsb = xs[s8]
        pooled = ps_pool.tile([128, 2, 80], F32, tag="pooled")
        wc4 = wcp.tile([128, 4, 2, 10], BF16, tag="wc4")
        nc.gpsimd.tensor_tensor(
            out=wc4[:, :, :, 0:8],
            in0=wtt[:, :, s8:64:8].unsqueeze(2).broadcast_to([128, 4, 2, 8]),
            in1=blockmask[:, :, 0:8].unsqueeze(1).broadcast_to([128, 4, 2, 8]),
            op=OP.mult)
        nc.gpsimd.tensor_tensor(
            out=wc4[:, :, :, 8:9],
            in0=wtt[:, :, 64 + s8:65 + s8]
            .unsqueeze(2).broadcast_to([128, 4, 2, 1]),
            in1=blockmask[:, :, 8:9].unsqueeze(1).broadcast_to([128, 4, 2, 1]),
            op=OP.mult)
        nc.gpsimd.tensor_tensor(
            out=wc4[:, :, :, 9:10],
            in0=wtt[:, :, 72 + s8:73 + s8]
            .unsqueeze(2).broadcast_to([128, 4, 2, 1]),
            in1=blockmask[:, :, 9:10].unsqueeze(1).broadcast_to([128, 4, 2, 1]),
            op=OP.mult)
        for sec in range(CHUNKS):
            wcf = wc4[:, sec].rearrange("p a b -> p (a b)")
            for k in range(2):
                nc.tensor.matmul(
                    pooled[:, k, sec * 20:sec * 20 + 20],
                    xsb[:, sec, k * 128:(k + 1) * 128], wcf,
                    start=True, stop=True)
        for k in range(2):
            nc.vector.tensor_copy(
                out=staged_x[:, k, s * 80:(s + 1) * 80],
                in_=pooled[:, k, :])


def _post_body(
    nc, tc, graphs_pc, gchunks, dt, staged_x, staged_sf,
    ones1, posw, pos, posT, ps_po, ps_pz, out_d, flags,
):
    cb0 = flags.get("cb0", False)
    cg1 = flags.get("cg1", False)
    cbt0 = flags.get("cbt0", False)
    hb0 = flags.get("hb0", False)
    hg1 = flags.get("hg1", False)
    hbt0 = flags.get("hbt0", False)
    p1b0 = flags.get("p1b0", False)
    p1g1 = flags.get("p1g1", False)
    p1bt0 = flags.get("p1bt0", False)
    p2b0 = flags.get("p2b0", False)

    def pload(name, shape, dtype=BF16):
        t = posw.tile(shape, dtype, tag=name, name=name)
        nc.sync.dma_start(out=t, in_=dt[name].ap())
        return t

    cw = pload("cw", [128, 16, 256])
    hw = pload("hw", [128, 4, 256])
    p1w = pload("p1w", [128, 6, 512])
    p2w = pload("p2w", [128, 4, 256])
    cbR = None if cb0 else pload("cb", [1, 256])
    hbR = None if hb0 else pload("hb", [1, 256])
    p1bR = None if p1b0 else pload("p1b", [1, 512])
    p2bR = None if p2b0 else pload("p2b", [1, 256])

    def bc(name, cols, skip):
        if skip:
            return None
        t = posw.tile([128, cols], F32, tag=f"{name}B", name=f"{name}B")
        _bcast(nc, t, dt[name])
        return t

    cgB = bc("cg", 256, cg1)
    cbtB = bc("cbt", 256, cbt0)
    hgB = bc("hg", 256, hg1)
    hbtB = bc("hbt", 256, hbt0)
    p1gB = bc("p1g", 512, p1g1)
    p1btB = bc("p1bt", 512, p1bt0)

    sx3 = staged_x.rearrange("p k (g t) -> p k g t", t=10)

    catT = posT.tile([128, 4, graphs_pc], BF16, tag="catT")
    zT = posT.tile([128, 4, graphs_pc], BF16, tag="zT")
    pmv = posT.tile([128, 2 * gchunks, 2], F32, tag="pmv")

    cps = []
    for gc in range(gchunks):
        gsl = slice(gc * 128, (gc + 1) * 128)
        cpp = ps_po.tile([128, 256], F32, tag="cpp")
        for h in range(H):
            for k in range(2):
                nc.tensor.matmul(
                    cpp, sx3[:, k, gsl, h], cw[:, h * 2 + k, :],
                    start=(h == 0 and k == 0),
                    stop=(cb0 and h == 7 and k == 1))
        if not cb0:
            nc.tensor.matmul(cpp, ones1, cbR, start=False, stop=True)
        hpp = ps_po.tile([128, 256], F32, tag="cpp", name="hpp")
        for k in range(2):
            nc.tensor.matmul(
                hpp, sx3[:, k, gsl, 8], hw[:, k, :],
                start=(k == 0), stop=False)
            nc.tensor.matmul(
                hpp, sx3[:, k, gsl, 9], hw[:, 2 + k, :],
                start=False, stop=(hb0 and k == 1))
        if not hb0:
            nc.tensor.matmul(hpp, ones1, hbR, start=False, stop=True)
        csb = posT.tile([128, 256], F32, tag=f"csb{gc}", name=f"csb{gc}")
        nc.vector.tensor_copy(out=csb, in_=cpp)
        hsb = posT.tile([128, 256], F32, tag=f"hsb{gc}", name=f"hsb{gc}")
        nc.scalar.activation(out=hsb, in_=hpp, func=AF.Copy)
        for i, ppx in enumerate((csb, hsb)):
            st6 = pos.tile([128, 6], F32, tag="pst6")
            nc.vector.bn_stats(out=st6, in_=ppx)
            nc.vector.bn_aggr(out=pmv[:, gc * 2 + i, :], in_=st6)
        cps.append((csb, hsb))

    prr = posT.tile([128, 2 * gchunks], F32, tag="prr")
    nc.vector.tensor_scalar(
        out=prr, in0=pmv[:, :, 1], scalar1=1.0, scalar2=1e-5,
        op0=OP.mult, op1=OP.add)
    nc.scalar.activation(out=prr, in_=prr, func=AF.Ln)
    nc.scalar.activation(out=prr, in_=prr, func=AF.Exp, scale=-0.5)

    for gc in range(gchunks):
        gsl = slice(gc * 128, (gc + 1) * 128)
        for i, (ppx, ggB, bbB, g1, bt0) in enumerate((
            (cps[gc][0], cgB, cbtB, cg1, cbt0),
            (cps[gc][1], hgB, hbtB, hg1, hbt0),
        )):
            mcol = pmv[:, gc * 2 + i, 0:1]
            rcol = prr[:, gc * 2 + i:gc * 2 + i + 1]
            rg = pos.tile([128, 256], BF16, tag="prg")
            if g1 and bt0:
                tg = pos.tile([128, 256], F32, tag="ptg")
                nc.vector.tensor_scalar(
                    out=tg, in0=ppx, scalar1=mcol, scalar2=rcol,
                    op0=OP.subtract, op1=OP.mult)
                nc.vector.tensor_scalar(
                    out=rg, in0=tg, scalar1=0.0, scalar2=None, op0=OP.max)
            else:
                tg = pos.tile([128, 256], F32, tag="ptg")
                if g1:
                    nc.vector.tensor_scalar(
                        out=tg, in0=ppx, scalar1=mcol, scalar2=rcol,
                        op0=OP.subtract, op1=OP.mult)
                else:
                    nc.vector.scalar_tensor_tensor(
                        out=tg, in0=ppx, scalar=mcol,
                        in1=ggB, op0=OP.subtract, op1=OP.mult)
                    nc.vector.tensor_scalar(
                        out=tg, in0=tg, scalar1=rcol, scalar2=None,
                        op0=OP.mult)
                if not bt0:
                    nc.vector.tensor_tensor(
                        out=tg, in0=tg, in1=bbB, op=OP.add)
                nc.vector.tensor_scalar(
                    out=rg, in0=tg, scalar1=0.0, scalar2=None, op0=OP.max)
            nc.sync.dma_start_transpose(
                out=catT[:, 2 * i:2 * i + 2, gsl], in_=rg)

    # p1 matmul + LN + relu -> zT
    pmv2 = posT.tile([128, gchunks, 2], F32, tag="pmv2")
    zpps = []
    for gc in range(gchunks):
        gsl = slice(gc * 128, (gc + 1) * 128)
        zpp = ps_pz.tile([128, 512], F32, tag="zpp")
        for kk in range(4):
            nc.tensor.matmul(
                zpp, catT[:, kk, gsl], p1w[:, kk, :],
                start=(kk == 0), stop=False)
        for kk in range(2):
            nc.tensor.matmul(
                zpp, staged_sf[:, kk, gsl], p1w[:, 4 + kk, :],
                start=False, stop=(p1b0 and kk == 1))
        if not p1b0:
            nc.tensor.matmul(zpp, ones1, p1bR, start=False, stop=True)
        zsb = posT.tile([128, 512], F32, tag=f"zsb{gc}", name=f"zsb{gc}")
        nc.vector.tensor_copy(out=zsb, in_=zpp)
        st6 = pos.tile([128, 6], F32, tag="pst6")
        nc.vector.bn_stats(out=st6, in_=zsb)
        nc.vector.bn_aggr(out=pmv2[:, gc, :], in_=st6)
        zpps.append(zsb)

    prr2 = posT.tile([128, gchunks], F32, tag="prr2")
    nc.vector.tensor_scalar(
        out=prr2, in0=pmv2[:, :, 1], scalar1=1.0, scalar2=1e-5,
        op0=OP.mult, op1=OP.add)
    nc.scalar.activation(out=prr2, in_=prr2, func=AF.Ln)
    nc.scalar.activation(out=prr2, in_=prr2, func=AF.Exp, scale=-0.5)

    for gc in range(gchunks):
        gsl = slice(gc * 128, (gc + 1) * 128)
        zsb = zpps[gc]
        mcol = pmv2[:, gc, 0:1]
        rcol = prr2[:, gc:gc + 1]
        rg = pos.tile([128, 512], BF16, tag="prg5")
        if p1g1 and p1bt0:
            tg = pos.tile([128, 512], F32, tag="ptg5")
            nc.vector.tensor_scalar(
                out=tg, in0=zsb, scalar1=mcol, scalar2=rcol,
                op0=OP.subtract, op1=OP.mult)
            nc.vector.tensor_scalar(
                out=rg, in0=tg, scalar1=0.0, scalar2=None, op0=OP.max)
        else:
            tg = pos.tile([128, 512], F32, tag="ptg5")
            if p1g1:
                nc.vector.tensor_scalar(
                    out=tg, in0=zsb, scalar1=mcol, scalar2=rcol,
                    op0=OP.subtract, op1=OP.mult)
            else:
                nc.vector.scalar_tensor_tensor(
                    out=tg, in0=zsb, scalar=mcol,
                    in1=p1gB, op0=OP.subtract, op1=OP.mult)
                nc.vector.tensor_scalar(
                    out=tg, in0=tg, scalar1=rcol, scalar2=None, op0=OP.mult)
            if not p1bt0:
                nc.vector.tensor_tensor(out=tg, in0=tg, in1=p1btB, op=OP.add)
            nc.vector.tensor_scalar(
                out=rg, in0=tg, scalar1=0.0, scalar2=None, op0=OP.max)
        nc.sync.dma_start_transpose(out=zT[:, :, gsl], in_=rg)

    # final projection
    for gc in range(gchunks):
        gsl = slice(gc * 128, (gc + 1) * 128)
        opp = ps_po.tile([128, 256], F32, tag="cpp", name="opp")
        for kk in range(4):
            nc.tensor.matmul(
                opp, zT[:, kk, gsl], p2w[:, kk, :],
                start=(kk == 0), stop=(p2b0 and kk == 3))
        if not p2b0:
            nc.tensor.matmul(opp, ones1, p2bR, start=False, stop=True)
        osb = pos.tile([128, 256], F32, tag="osb")
        nc.scalar.activation(out=osb, in_=opp, func=AF.Copy)
        nc.sync.dma_start(out=out_d.ap()[gsl, :], in_=osb)


# ---------------------------------------------------------------------------
# host side
# ---------------------------------------------------------------------------

_NC_CACHE = {}


def _get_nc(n_mega=FULL_N_MEGA, flags=None, mm_mode=MM_MODE, repeat=1):
    flags = flags or {}
    key = (n_mega, tuple(sorted(flags.items())), mm_mode, SIM_SAFE, repeat)
    if key not in _NC_CACHE:
        _NC_CACHE[key] = build_nc(n_mega, flags, mm_mode, repeat=repeat)
    return _NC_CACHE[key]


def _bf(a):
    return np.ascontiguousarray(np.asarray(a, np.float32).astype(
        ml_dtypes.bfloat16))


def _f8(a):
    return np.ascontiguousarray(np.asarray(a, np.float32).astype(
        ml_dtypes.float8_e4m3))


def _detect_flags(inp):
    f = {}

    def allz(k):
        return bool((np.asarray(inp[k]) == 0).all())

    def all1(k):
        return bool((np.asarray(inp[k]) == 1).all())

    f["spb0"] = allz("sp_b")
    f["spg1"] = all1("sp_g")
    f["spbt0"] = allz("sp_beta")
    f["b20"] = (allz("att_b2") and allz("piece_b2") and allz("empty_b2"))
    f["cb0"] = allz("comb_b")
    f["cg1"] = all1("comb_g")
    f["cbt0"] = allz("comb_beta")
    f["hb0"] = allz("hier_b")
    f["hg1"] = all1("hier_g")
    f["hbt0"] = allz("hier_beta")
    f["p1b0"] = allz("p1_b")
    f["p1g1"] = all1("p1_g")
    f["p1bt0"] = allz("p1_beta")
    f["p2b0"] = allz("p2_b")
    return f


def _prep_weights(inp, mm_mode=MM_MODE):
    f = np.float32
    att_W1 = np.asarray(inp["att_W1"], f)          # [8, 256, 64]
    att_b1 = np.asarray(inp["att_b1"], f)          # [8, 64]
    att_w2 = np.asarray(inp["att_w2"], f)          # [8, 64]
    piece_W1 = np.asarray(inp["piece_W1"], f)      # [256, 128]
    empty_W1 = np.asarray(inp["empty_W1"], f)
    piece_b1 = np.asarray(inp["piece_b1"], f)      # [128]
    empty_b1 = np.asarray(inp["empty_b1"], f)
    piece_w2 = np.asarray(inp["piece_w2"], f)      # [128]
    empty_w2 = np.asarray(inp["empty_w2"], f)

    w1cat = np.transpose(att_W1, (1, 0, 2)).reshape(256, 512)  # [c, h*64+d]
    w1 = w1cat.reshape(2, 128, 512).transpose(1, 0, 2)         # [p, k, col]
    b1a = np.ascontiguousarray(att_b1.reshape(512).reshape(4, 128).T)
    pecat = np.concatenate([piece_W1, empty_W1], 1)            # [256, 256]
    pew = pecat.reshape(2, 128, 256).transpose(1, 0, 2)
    b1p = np.ascontiguousarray(
        np.concatenate([piece_b1, empty_b1]).reshape(2, 128).T)
    w2c = np.zeros((128, 4, 16), f)
    for h in range(H):
        m, half = divmod(h, 2)
        w2c[64 * half:64 * (half + 1), m, h] = att_w2[h]
    pw2c = np.zeros((128, 2, 16), f)
    pw2c[:, 0, 8] = piece_w2
    pw2c[:, 1, 9] = empty_w2
    b2col = np.zeros((80, 1), f)
    att_b2 = np.asarray(inp["att_b2"], f)
    for h in range(8):
        b2col[h * 8:h * 8 + 8, 0] = att_b2[h]
    b2col[64:72, 0] = np.float32(inp["piece_b2"])
    b2col[72:80, 0] = np.float32(inp["empty_b2"])
    spW = np.asarray(inp["sp_W"], np.float64)
    spW = spW - spW.mean(axis=1, keepdims=True)   # exact LN centering
    spwm = spW.astype(f).reshape(2, 128, 256).transpose(1, 0, 2)
    sav = (1.0 / (1.0 + np.exp(-np.asarray(inp["strat_w"], np.float64))))
    sav = np.tile(sav.reshape(64), 2).astype(f).reshape(128, 1)
    c = np.ascontiguousarray
    wd = {
        "w1": _bf(w1), "b1a": c(b1a), "pew": _bf(pew), "b1p": c(b1p),
        "w2c": _bf(w2c), "pw2c": _bf(pw2c), "b2col": c(b2col),
        "spw": _bf(spwm),
        "spb": _bf((np.asarray(inp["sp_b"], np.float64)
                    - np.asarray(inp["sp_b"], np.float64).mean())
                   .astype(f).reshape(1, 256)),
        "spg": c(np.asarray(inp["sp_g"], f).reshape(1, 256)),
        "spbt": c(np.asarray(inp["sp_beta"], f).reshape(1, 256)),
        "sa": c(sav), "sa2": c(sav * sav),
        "cw": _bf(np.asarray(inp["comb_W"], f).reshape(16, 128, 256)
                  .transpose(1, 0, 2)),
        "cb": _bf(np.asarray(inp["comb_b"], f).reshape(1, 256)),
        "cg": c(np.asarray(inp["comb_g"], f).reshape(1, 256)),
        "cbt": c(np.asarray(inp["comb_beta"], f).reshape(1, 256)),
        "hw": _bf(np.asarray(inp["hier_W"], f).reshape(4, 128, 256)
                  .transpose(1, 0, 2)),
        "hb": _bf(np.asarray(inp["hier_b"], f).reshape(1, 256)),
        "hg": c(np.asarray(inp["hier_g"], f).reshape(1, 256)),
        "hbt": c(np.asarray(inp["hier_beta"], f).reshape(1, 256)),
        "p1w": _bf(np.asarray(inp["p1_W"], f).reshape(6, 128, 512)
                   .transpose(1, 0, 2)),
        "p1b": _bf(np.asarray(inp["p1_b"], f).reshape(1, 512)),
        "p1g": c(np.asarray(inp["p1_g"], f).reshape(1, 512)),
        "p1bt": c(np.asarray(inp["p1_beta"], f).reshape(1, 512)),
        "p2w": _bf(np.asarray(inp["p2_W"], f).reshape(4, 128, 256)
                   .transpose(1, 0, 2)),
        "p2b": _bf(np.asarray(inp["p2_b"], f).reshape(1, 256)),
    }
    if mm_mode == "fp8":
        wd["w1q"] = _f8(w1)
        wd["pewq"] = _f8(pew)
        wd["spwq"] = _f8(spwm)
        # att L2 stationary in DR pairs: [p, i(k of hL pair), mpair, col]
        w2cq = np.zeros((128, 2, 2, 16), f)
        w2cq[:, 0, 0, :] = w2c[:, 0, :]
        w2cq[:, 1, 0, :] = w2c[:, 1, :]
        w2cq[:, 0, 1, :] = w2c[:, 2, :]
        w2cq[:, 1, 1, :] = w2c[:, 3, :]
        wd["w2cq"] = _f8(w2cq)
        wd["pw2cq"] = _f8(pw2c)
    return wd


def make_in_maps(inputs, n_mega=FULL_N_MEGA, mm_mode=MM_MODE):
    x = np.asarray(inputs["x"], np.float32)
    nt = np.asarray(inputs["node_types"])
    mb = (nt > 0).astype(ml_dtypes.bfloat16)
    nb = (nt <= 0).astype(ml_dtypes.bfloat16)
    xb = x.astype(ml_dtypes.bfloat16)
    wd = _prep_weights(inputs, mm_mode)
    nodes_pc = n_mega * MEGA * ST
    in_maps = []
    for cc in range(N_CORES):
        sl = slice(cc * nodes_pc, (cc + 1) * nodes_pc)
        m = {"x": np.ascontiguousarray(xb[sl]),
             "mb": np.ascontiguousarray(mb[sl]),
             "nb": np.ascontiguousarray(nb[sl])}
        m.update(wd)
        in_maps.append(m)
    return in_maps


def run(inputs, n_mega=FULL_N_MEGA, mm_mode=MM_MODE):
    flags = _detect_flags(inputs)
    nc = _get_nc(n_mega, flags, mm_mode)
    in_maps = make_in_maps(inputs, n_mega, mm_mode)
    res = run_bass_kernel_spmd(nc, in_maps, core_ids=list(range(N_CORES)))
    return np.concatenate(
        [res.results[cc]["out"] for cc in range(N_CORES)], axis=0
    )


def kernel(**inputs):
    return run(inputs, FULL_N_MEGA)


# revision 25
# speedup vs baseline: 1.2380x; 1.2380x over previous
"""Trainium2 Bass kernel for nn_ChessGraphPooling (segment_reduce).

Data-parallel over whole graphs: 4096 boards x 64 nodes sharded across 8
NeuronCores (512 graphs / 32768 nodes per core); small weights replicated.

v3 design (bf16 pipeline, engine-balanced):
  - x converted to bf16 on host; per-core DMA traffic halves.
  - node-layout x tiles feed pooling matmuls as stationary operands and are
    transposed into channel-major T-layout [c, nodes] by the DMA engines'
    XBAR transpose (zero PE/DVE cost).
  - scorer linears run on the PE in T-layout; hidden activations (LeakyRelu)
    on the scalar engine; tiny second layers are [10,512] matmuls.
  - segment softmax skips the max-subtract pass (exact: softmax is
    shift-invariant and scores are O(1), exp cannot overflow); scores are
    stacked per-mega (8 supertiles, 80 rows) so every softmax op is one
    free-size-bound pass.
  - all rsqrt = Exp(-0.5*Ln(x)) so the ACT engine keeps one activation
    table (Prelu/Exp/Ln/Square live in natural_log_exp_and_others) and never
    pays the 1.3us table reload.
  - strategic branch LN runs in node layout via bn_stats; normalize+relu is
    a single fused tensor_scalar (sub,mult / mult,max) when gamma==1/beta==0
    (detected on host, the general path adds the row ops back).
  - pooling writes T-layout directly: stationary = x/sf node-layout chunks,
    moving = per-graph weight columns; pooled results land in PSUM already
    channel-major, so no pool transposes and no re-staging.
  - post stage (per-graph MLPs) reuses the same tricks: bf16 matmuls, DMA
    transposes, Pool-engine PSUM drains.
"""

import os
import sys

sys.path.insert(0, "/opt/trn_rl_repo")

from contextlib import ExitStack

import numpy as np
import ml_dtypes

import concourse.bass as bass
import concourse.bacc as bacc
import concourse.tile as tile
import concourse.mybir as mybir
from concourse.bass_utils import run_bass_kernel_spmd

F32 = mybir.dt.float32
BF16 = mybir.dt.bfloat16
FP8 = mybir.dt.float8e4
I32 = mybir.dt.int32
AF = mybir.ActivationFunctionType
OP = mybir.AluOpType
AX = mybir.AxisListType
DRMODE = mybir.MatmulPerfMode.DoubleRow

C = 256
H = 8
NODES = 64
NEG = 0.2
N_CORES = 8
ST = 512          # nodes per supertile
CHUNKS = 4        # 128-node chunks per supertile
MEGA = 8          # supertiles per megatile (80 score rows)
FULL_N_MEGA = 8   # megatiles per core at full size

# matmul input mode: "bf16" or "fp8" (DoubleRow) for the scorer/strat linears
MM_MODE = os.environ.get("K_MM_MODE", "fp8")
# CoreSim does not implement Prelu; sim_safe swaps it for an Abs-based
# decomposition (0.6*v + 0.4*|v|) with identical numerics
SIM_SAFE = bool(os.environ.get("K_SIM_SAFE"))


def build_nc(n_mega=FULL_N_MEGA, flags=None, mm_mode=MM_MODE,
             sim_safe=None, repeat=1):
    flags = dict(flags or {})
    flags["_sim_safe"] = SIM_SAFE if sim_safe is None else sim_safe
    nodes_pc = n_mega * MEGA * ST
    graphs_pc = nodes_pc // NODES
    assert graphs_pc % 128 == 0

    nc = bacc.Bacc("TRN2", num_devices=N_CORES)

    dt = {}

    def din(name, shape, dtype=F32):
        dt[name] = nc.dram_tensor(name, shape, dtype, kind="ExternalInput")

    din("x", [nodes_pc, C], BF16)
    din("mb", [nodes_pc], BF16)       # piece mask (node_types > 0)
    din("nb", [nodes_pc], BF16)       # 1 - piece mask
    din("w1", [128, 2, 512], BF16)    # att W1, [p, k, h*64+d]
    din("b1a", [128, 4])              # att b1 cols per m-chunk
    din("pew", [128, 2, 256], BF16)   # piece|empty W1
    din("b1p", [128, 2])
    din("w2c", [128, 4, 16], BF16)    # att w2 -> score rows 0..7
    din("pw2c", [128, 2, 16], BF16)   # piece/empty w2 -> rows 8,9
    din("b2col", [80, 1])
    din("spw", [128, 2, 256], BF16)
    din("spb", [1, 256], BF16)
    din("spg", [1, 256])
    din("spbt", [1, 256])
    din("sa", [128, 1])
    din("sa2", [128, 1])
    din("cw", [128, 16, 256], BF16)
    din("cb", [1, 256], BF16)
    din("cg", [1, 256])
    din("cbt", [1, 256])
    din("hw", [128, 4, 256], BF16)
    din("hb", [1, 256], BF16)
    din("hg", [1, 256])
    din("hbt", [1, 256])
    din("p1w", [128, 6, 512], BF16)
    din("p1b", [1, 512], BF16)
    din("p1g", [1, 512])
    din("p1bt", [1, 512])
    din("p2w", [128, 4, 256], BF16)
    din("p2b", [1, 256], BF16)
    if mm_mode == "fp8":
        din("w1q", [128, 2, 512], FP8)
        din("pewq", [128, 2, 256], FP8)
        din("spwq", [128, 2, 256], FP8)
        din("w2cq", [128, 2, 2, 16], FP8)   # [p, i, mpair, col]
        din("pw2cq", [128, 2, 16], FP8)     # pe pair: [p, i(m), col]
    out_d = nc.dram_tensor("out", [graphs_pc, C], F32, kind="ExternalOutput")

    with tile.TileContext(nc) as tc:
        for _rep in range(repeat):
            _build_body(nc, tc, n_mega, graphs_pc, dt, out_d, flags, mm_mode)
    nc.compile()
    return nc


def _bcast(nc, dst, src_d):
    nc.gpsimd.dma_start(
        out=dst, in_=src_d.ap().partition_broadcast(dst.shape[0])
    )


def _build_body(nc, tc, n_mega, graphs_pc, dt, out_d, flags, mm_mode):
    gchunks = graphs_pc // 128
    fp8 = mm_mode == "fp8"

    nc.scalar.add_instruction(mybir.InstLoadActFuncSet(
        name=nc.get_next_instruction_name(), ins=[], outs=[],
        act_func_set_id=6))  # natural_log_exp_and_others: Prelu+Exp+Ln

    with ExitStack() as top:
        consts = top.enter_context(tc.tile_pool(name="consts", bufs=1))
        persist = top.enter_context(tc.tile_pool(name="persist", bufs=1))

        def cload(name, shape, dtype=F32, src=None):
            t = consts.tile(shape, dtype, tag=name, name=name)
            nc.sync.dma_start(out=t, in_=dt[src or name].ap())
            return t

        w1 = cload("w1", [128, 2, 512], BF16)
        b1a = cload("b1a", [128, 4])
        pew = cload("pew", [128, 2, 256], BF16)
        b1p = cload("b1p", [128, 2])
        w2c = cload("w2c", [128, 4, 16], BF16)
        pw2c = cload("pw2c", [128, 2, 16], BF16)
        b2col = cload("b2col", [80, 1])
        spw = cload("spw", [128, 2, 256], BF16)
        sa = cload("sa", [128, 1])
        sa2 = cload("sa2", [128, 1])
        if fp8:
            w1q = cload("w1q", [128, 2, 512], FP8)
            pewq = cload("pewq", [128, 2, 256], FP8)
            spwq = cload("spwq", [128, 2, 256], FP8)
            w2cq = cload("w2cq", [128, 2, 2, 16], FP8)
            pw2cq = cload("pw2cq", [128, 2, 16], FP8)

        ones1 = consts.tile([1, 128], BF16, tag="ones1")
        nc.vector.memset(ones1, 1.0)
        spbrow = None
        if not flags.get("spb0", False):
            spbrow = cload("spbrow", [1, 256], BF16, src="spb")
        spgB = spbtB = None
        if not flags.get("spg1", False):
            spgB = consts.tile([128, 256], F32, tag="spgB")
            _bcast(nc, spgB, dt["spg"])
        if not flags.get("spbt0", False):
            spbtB = consts.tile([128, 256], F32, tag="spbtB")
            _bcast(nc, spbtB, dt["spbt"])

        poolcol = consts.tile([128, 2], BF16, tag="poolcol")
        nc.vector.memset(poolcol, 0.0)
        nc.vector.memset(poolcol[0:64, 0:1], 1.0 / NODES)
        nc.vector.memset(poolcol[64:128, 1:2], 1.0 / NODES)
        blockmask = consts.tile([128, 2, 10], BF16, tag="blockmask")
        nc.vector.memset(blockmask, 0.0)
        nc.vector.memset(blockmask[0:64, 0, :], 1.0)
        nc.vector.memset(blockmask[64:128, 1, :], 1.0)
        # double-buffered mask stacks; att rows stay 1.0 forever
        maskS = []
        for i in range(2):
            mt = consts.tile([80, 512], BF16, tag=f"maskS{i}", name=f"maskS{i}")
            nc.vector.memset(mt, 1.0)
            maskS.append(mt)

        staged_x = persist.tile(
            [128, 2, graphs_pc * 10], BF16, tag="staged_x")
        staged_sf = persist.tile([128, 2, graphs_pc], BF16, tag="staged_sf")

        with ExitStack() as main:
            xpool = main.enter_context(tc.tile_pool(name="xpool", bufs=9))
            xTp = main.enter_context(tc.tile_pool(name="xTp", bufs=5))
            actp = main.enter_context(tc.tile_pool(name="actp", bufs=6))
            sfp = main.enter_context(tc.tile_pool(name="sfp", bufs=4))
            wcp = main.enter_context(tc.tile_pool(name="wcp", bufs=4))
            scr = main.enter_context(tc.tile_pool(name="scr", bufs=8))
            megap = main.enter_context(tc.tile_pool(name="megap", bufs=3))

            ps_mm = main.enter_context(
                tc.tile_pool(name="ps_mm", bufs=3, space="PSUM"))
            ps_sc = main.enter_context(
                tc.tile_pool(name="ps_sc", bufs=2, space="PSUM"))
            ps_pool = main.enter_context(
                tc.tile_pool(name="ps_pool", bufs=2, space="PSUM"))
            ps_sfp = main.enter_context(
                tc.tile_pool(name="ps_sfp", bufs=1, space="PSUM"))

            for mg in range(n_mega):
                _mega_body(
                    nc, tc, mg, dt, staged_x, staged_sf,
                    w1q if fp8 else w1, b1a, pewq if fp8 else pew, b1p,
                    w2cq if fp8 else w2c, pw2cq if fp8 else pw2c, b2col,
                    spwq if fp8 else spw, spbrow, spgB, spbtB, sa, sa2,
                    ones1, poolcol, blockmask, maskS[mg % 2],
                    xpool, xTp, actp, sfp, wcp, scr, megap,
                    ps_mm, ps_sc, ps_pool, ps_sfp, flags, fp8,
                )

        with ExitStack() as post:
            posw = post.enter_context(tc.tile_pool(name="posw", bufs=1))
            pos = post.enter_context(tc.tile_pool(name="pos", bufs=4))
            posT = post.enter_context(tc.tile_pool(name="posT", bufs=1))
            ps_po = post.enter_context(
                tc.tile_pool(name="ps_po", bufs=2, space="PSUM"))
            ps_pz = post.enter_context(
                tc.tile_pool(name="ps_pz", bufs=2, space="PSUM"))
            _post_body(
                nc, tc, graphs_pc, gchunks, dt, staged_x, staged_sf,
                ones1, posw, pos, posT, ps_po, ps_pz, out_d, flags,
            )


def _prelu(nc, scr, out, ph, bias_col, sim_safe):
    """out = LeakyRelu(ph + bias, NEG)."""
    if not sim_safe:
        nc.scalar.activation(
            out=out, in_=ph, func=AF.Prelu,
            bias=bias_col, scale=1.0, alpha=NEG)
        return
    shape = [ph.shape[0], ph.shape[-1]]
    absv = scr.tile(shape, F32, tag="pabs", name="pabs")
    nc.scalar.activation(
        out=absv, in_=ph, func=AF.Abs, bias=bias_col, scale=1.0)
    t = scr.tile(shape, F32, tag="pt", name="pt")
    nc.vector.tensor_scalar(
        out=t, in0=ph, scalar1=bias_col, scalar2=(1.0 + NEG) / 2.0,
        op0=OP.add, op1=OP.mult)
    nc.vector.scalar_tensor_tensor(
        out=out, in0=absv, scalar=(1.0 - NEG) / 2.0,
        in1=t, op0=OP.mult, op1=OP.add)


def _mega_body(
    nc, tc, mg, dt, staged_x, staged_sf,
    w1, b1a, pew, b1p, w2c, pw2c, b2col, spw, spbrow, spgB, spbtB, sa, sa2,
    ones1, poolcol, blockmask, maskS,
    xpool, xTp, actp, sfp, wcp, scr, megap,
    ps_mm, ps_sc, ps_pool, ps_sfp, flags, fp8,
):
    sim_safe = flags.get("_sim_safe", False)
    spb0 = flags.get("spb0", False)
    spg1 = flags.get("spg1", False)
    spbt0 = flags.get("spbt0", False)
    b20 = flags.get("b20", False)
    mmdt = FP8 if fp8 else BF16

    # per-mega pe-score masks: blocked rows (att 0:64, piece 64:72, empty 72:80)
    nc.sync.dma_start(
        out=maskS[64:72, :],
        in_=dt["mb"].ap()[mg * MEGA * ST:(mg + 1) * MEGA * ST]
        .rearrange("(s n) -> s n", s=8),
    )
    nc.sync.dma_start(
        out=maskS[72:80, :],
        in_=dt["nb"].ap()[mg * MEGA * ST:(mg + 1) * MEGA * ST]
        .rearrange("(s n) -> s n", s=8),
    )

    sstack = megap.tile([80, 512], BF16, tag="sstack")
    stbuf = megap.tile([16, 8, 512], BF16, tag="stbuf")
    xs = []
    xsb2 = None
    for s8 in range(MEGA):
        s = mg * MEGA + s8
        if s8 % 2 == 0:
            xsb2 = xpool.tile([128, 8, 256], BF16, tag="xsb")
            nc.sync.dma_start(
                out=xsb2,
                in_=dt["x"].ap()[s * ST:(s + 2) * ST, :]
                .rearrange("(c p) m -> p c m", p=128),
            )
        xsb = xsb2[:, (s8 % 2) * 4:(s8 % 2) * 4 + 4, :]
        xs.append(xsb)

        # one XBAR transpose per supertile over the flat [128,1024] view:
        # xTf[p, 2c+k, n] = x[node c*128+n, k*128+p]
        xTf = xTp.tile([128, 8, 128], BF16, tag="xT")
        nc.sync.dma_start_transpose(
            out=xTf, in_=xsb.rearrange("p c m -> p (c m)"))
        if fp8:
            # shuffle-convert to standard DR layout [p, k, c*128+n]
            xTq = xTp.tile([128, 2, 512], FP8, tag="xTq", name="xTq")
            eng = nc.gpsimd
            eng.tensor_copy(
                out=xTq.rearrange("p k (c n) -> p k c n", n=128),
                in_=xTf.rearrange("p (c k) n -> p k c n", k=2))
            xmv = xTq
        else:
            xmv = xTf.rearrange("p (c k) n -> p k c n", k=2)

        scp = ps_sc.tile([16, 512], F32, tag="scp")
        # attention scorer
        hLt = None
        for m in range(4):
            ph = ps_mm.tile([128, 512], F32, tag="ph")
            if fp8:
                nc.tensor.matmul(
                    ph, w1[:, :, m * 128:(m + 1) * 128], xmv,
                    start=True, stop=True, perf_mode=DRMODE)
            else:
                nc.tensor.matmul(
                    ph, w1[:, 0, m * 128:(m + 1) * 128], xmv[:, 0],
                    start=True, stop=False)
                nc.tensor.matmul(
                    ph, w1[:, 1, m * 128:(m + 1) * 128], xmv[:, 1],
                    start=False, stop=True)
            if m % 2 == 0:
                hLt = actp.tile([128, 2, 512], mmdt, tag="hL")
            _prelu(nc, scr, hLt[:, m % 2, :], ph, b1a[:, m:m + 1], sim_safe)
            if fp8:
                if m % 2 == 1:
                    nc.tensor.matmul(
                        scp[0:10, :], w2c[:, :, m // 2, 0:10], hLt,
                        start=(m == 1), stop=False, perf_mode=DRMODE)
            else:
                nc.tensor.matmul(
                    scp[0:10, :], w2c[:, m, 0:10], hLt[:, m % 2, :],
                    start=(m == 0), stop=False)

        # piece/empty scorer
        peLt = actp.tile([128, 2, 512], mmdt, tag="peL", name="peL")
        for m in range(2):
            pp = ps_mm.tile([128, 512], F32, tag="ph", name="pp")
            if fp8:
                nc.tensor.matmul(
                    pp, pew[:, :, m * 128:(m + 1) * 128], xmv,
                    start=True, stop=True, perf_mode=DRMODE)
            else:
                nc.tensor.matmul(
                    pp, pew[:, 0, m * 128:(m + 1) * 128], xmv[:, 0],
                    start=True, stop=False)
                nc.tensor.matmul(
                    pp, pew[:, 1, m * 128:(m + 1) * 128], xmv[:, 1],
                    start=False, stop=True)
            _prelu(nc, scr, peLt[:, m, :], pp, b1p[:, m:m + 1], sim_safe)
        if fp8:
            nc.tensor.matmul(
                scp[0:10, :], pw2c[:, :, 0:10], peLt,
                start=False, stop=True, perf_mode=DRMODE)
        else:
            for m in range(2):
                nc.tensor.matmul(
                    scp[0:10, :], pw2c[:, m, 0:10], peLt[:, m, :],
                    start=False, stop=(m == 1))

        # drain scores into the SBUF stage buffer (no DMA)
        nc.vector.tensor_copy(out=stbuf[0:10, s8, :], in_=scp[0:10, :])

        # strategic branch: z' = x @ (spW - rowmean(spW)) is exactly
        # centered, so LN needs only the variance; rho is computed
        # per-supertile and the PSUM drain fuses scale+relu in one op.
        mv4 = scr.tile([128, 4, 2], F32, tag="mv4", name="mv4")
        pzs = []
        for cp in range(2):
            pz = ps_mm.tile([128, 512], F32, tag="ph", name="pz")
            pzs.append(pz)
            for half in range(2):
                sec = cp * 2 + half
                sl = pz[:, half * 256:(half + 1) * 256]
                last = spb0
                if fp8:
                    nc.tensor.matmul(
                        sl, xmv[:, :, sec * 128:(sec + 1) * 128], spw,
                        start=True, stop=last, perf_mode=DRMODE)
                else:
                    nc.tensor.matmul(
                        sl, xmv[:, 0, sec], spw[:, 0, :],
                        start=True, stop=False)
                    nc.tensor.matmul(
                        sl, xmv[:, 1, sec], spw[:, 1, :],
                        start=False, stop=last)
                if not spb0:
                    nc.tensor.matmul(sl, ones1, spbrow, start=False, stop=True)
            for half in range(2):
                sec = cp * 2 + half
                sl = pz[:, half * 256:(half + 1) * 256]
                st6 = scr.tile([128, 6], F32, tag="st6")
                nc.vector.bn_stats(out=st6, in_=sl)
                nc.vector.bn_aggr(out=mv4[:, sec, :], in_=st6)
        rho4 = scr.tile([128, 4], F32, tag="rho4", name="rho4")
        nc.vector.tensor_scalar(
            out=rho4, in0=mv4[:, :, 1], scalar1=sa2, scalar2=1e-5,
            op0=OP.mult, op1=OP.add)
        nc.scalar.activation(out=rho4, in_=rho4, func=AF.Ln)
        nc.scalar.activation(out=rho4, in_=rho4, func=AF.Exp, scale=-0.5)
        nc.vector.tensor_scalar(
            out=rho4, in0=rho4, scalar1=sa, scalar2=None, op0=OP.mult)
        sff = sfp.tile([128, 4, 256], BF16, tag="sf")
        for cp in range(2):
            pz = pzs[cp]
            for half in range(2):
                sec = cp * 2 + half
                sl = pz[:, half * 256:(half + 1) * 256]
                rcol = rho4[:, sec:sec + 1]
                if spg1 and spbt0:
                    if sec != 0:
                        nc.vector.tensor_scalar(
                            out=sff[:, sec, :], in0=sl, scalar1=rcol,
                            scalar2=0.0, op0=OP.mult, op1=OP.max)
                    else:
                        nc.scalar.activation(
                            out=sff[:, sec, :], in_=sl, func=AF.Relu,
                            scale=rcol, bias=0.0)
                else:
                    tg = scr.tile([128, 256], F32, tag="tg", name="tg")
                    nc.vector.tensor_scalar(
                        out=tg, in0=sl, scalar1=rcol, scalar2=None,
                        op0=OP.mult)
                    if spgB is not None:
                        nc.vector.tensor_tensor(
                            out=tg, in0=tg, in1=spgB, op=OP.mult)
                    if spbtB is not None:
                        nc.vector.tensor_tensor(
                            out=tg, in0=tg, in1=spbtB, op=OP.add)
                    nc.vector.tensor_scalar(
                        out=sff[:, sec, :], in0=tg, scalar1=0.0,
                        scalar2=None, op0=OP.max)
        # sf mean-pool straight to T-layout and stage it (phase A)
        psf = ps_sfp.tile([128, 2, 8], F32, tag="psf")
        for sec in range(CHUNKS):
            for k in range(2):
                nc.tensor.matmul(
                    psf[:, k, sec * 2:sec * 2 + 2],
                    sff[:, sec, k * 128:(k + 1) * 128], poolcol,
                    start=True, stop=True)
        for k in range(2):
            nc.vector.tensor_copy(
                out=staged_sf[:, k, s * 8:(s + 1) * 8], in_=psf[:, k, :])

    # assemble the mega score stack: 3 DMAs
    nc.sync.dma_start(out=sstack[0:64, :], in_=stbuf[0:8])
    nc.sync.dma_start(out=sstack[64:72, :], in_=stbuf[8:9])
    nc.sync.dma_start(out=sstack[72:80, :], in_=stbuf[9:10])

    # ---- phase B: batched segment softmax (no max-subtract; exact) ----
    if b20:
        nc.gpsimd.tensor_tensor(
            out=sstack, in0=sstack, in1=maskS, op=OP.mult)
    else:
        nc.gpsimd.scalar_tensor_tensor(
            out=sstack, in0=sstack, scalar=b2col,
            in1=maskS, op0=OP.add, op1=OP.mult)
    estack = megap.tile([80, 512], BF16, tag="estack")
    nc.scalar.activation(out=estack, in_=sstack, func=AF.Exp)
    dsum = megap.tile([80, 8], F32, tag="dsum")
    nc.vector.tensor_reduce(
        out=dsum, in_=estack.rearrange("p (g n) -> p g n", n=NODES),
        axis=AX.X, op=OP.add)
    nc.vector.tensor_scalar(
        out=dsum, in0=dsum, scalar1=1e-16, scalar2=None, op0=OP.add)
    dre = megap.tile([80, 8], BF16, tag="dre")
    with nc.allow_low_precision(reason="softmax denom reciprocal, bf16 ok"):
        nc.vector.reciprocal(out=dre, in_=dsum)
    wT = megap.tile([80, 512], BF16, tag="wT")
    nc.gpsimd.tensor_tensor(
        out=wT.rearrange("p (g n) -> p g n", n=NODES),
        in0=estack.rearrange("p (g n) -> p g n", n=NODES),
        in1=dre.unsqueeze(2).broadcast_to([80, 8, NODES]),
        op=OP.mult)
    # wtt[p, c, r] = wT[r, c*128+p]
    wtt = megap.tile([128, 4, 80], BF16, tag="wtt")
    nc.sync.dma_start_transpose(out=wtt, in_=wT)

    # ---- phase C: attention/piece/empty pooling ----
    for s8 in range(MEGA):
        s = mg * MEGA + s8
        xsb = xs[s8]
        pooled = ps_pool.tile([128, 2, 80], F32, tag="pooled")
        wc4 = wcp.tile([128, 4, 2, 10], BF16, tag="wc4")
        nc.gpsimd.tensor_tensor(
            out=wc4[:, :, :, 0:8],
            in0=wtt[:, :, s8:64:8].unsqueeze(2).broadcast_to([128, 4, 2, 8]),
            in1=blockmask[:, :, 0:8].unsqueeze(1).broadcast_to([128, 4, 2, 8]),
            op=OP.mult)
        nc.gpsimd.tensor_tensor(
            out=wc4[:, :, :, 8:9],
            in0=wtt[:, :, 64 + s8:65 + s8]
            .unsqueeze(2).broadcast_to([128, 4, 2, 1]),
            in1=blockmask[:, :, 8:9].unsqueeze(1).broadcast_to([128, 4, 2, 1]),
            op=OP.mult)
        nc.gpsimd.tensor_tensor(
            out=wc4[:, :, :, 9:10],
            in0=wtt[:, :, 72 + s8:73 + s8]
            .unsqueeze(2).broadcast_to([128, 4, 2, 1]),
            in1=blockmask[:, :, 9:10].unsqueeze(1).broadcast_to([128, 4, 2, 1]),
            op=OP.mult)
        for sec in range(CHUNKS):
            wcf = wc4[:, sec].rearrange("p a b -> p (a b)")
            for k in range(2):
                nc.tensor.matmul(
                    pooled[:, k, sec * 20:sec * 20 + 20],
                    xsb[:, sec, k * 128:(k + 1) * 128], wcf,
                    start=True, stop=True)
        for k in range(2):
            nc.vector.tensor_copy(
                out=staged_x[:, k, s * 80:(s + 1) * 80],
                in_=pooled[:, k, :])


def _post_body(
    nc, tc, graphs_pc, gchunks, dt, staged_x, staged_sf,
    ones1, posw, pos, posT, ps_po, ps_pz, out_d, flags,
):
    cb0 = flags.get("cb0", False)
    cg1 = flags.get("cg1", False)
    cbt0 = flags.get("cbt0", False)
    hb0 = flags.get("hb0", False)
    hg1 = flags.get("hg1", False)
    hbt0 = flags.get("hbt0", False)
    p1b0 = flags.get("p1b0", False)
    p1g1 = flags.get("p1g1", False)
    p1bt0 = flags.get("p1bt0", False)
    p2b0 = flags.get("p2b0", False)

    def pload(name, shape, dtype=BF16):
        t = posw.tile(shape, dtype, tag=name, name=name)
        nc.sync.dma_start(out=t, in_=dt[name].ap())
        return t

    cw = pload("cw", [128, 16, 256])
    hw = pload("hw", [128, 4, 256])
    p1w = pload("p1w", [128, 6, 512])
    p2w = pload("p2w", [128, 4, 256])
    cbR = None if cb0 else pload("cb", [1, 256])
    hbR = None if hb0 else pload("hb", [1, 256])
    p1bR = None if p1b0 else pload("p1b", [1, 512])
    p2bR = None if p2b0 else pload("p2b", [1, 256])

    def bc(name, cols, skip):
        if skip:
            return None
        t = posw.tile([128, cols], F32, tag=f"{name}B", name=f"{name}B")
        _bcast(nc, t, dt[name])
        return t

    cgB = bc("cg", 256, cg1)
    cbtB = bc("cbt", 256, cbt0)
    hgB = bc("hg", 256, hg1)
    hbtB = bc("hbt", 256, hbt0)
    p1gB = bc("p1g", 512, p1g1)
    p1btB = bc("p1bt", 512, p1bt0)

    sx3 = staged_x.rearrange("p k (g t) -> p k g t", t=10)

    catT = posT.tile([128, 4, graphs_pc], BF16, tag="catT")
    zT = posT.tile([128, 4, graphs_pc], BF16, tag="zT")
    pmv = posT.tile([128, 2 * gchunks, 2], F32, tag="pmv")

    cps = []
    for gc in range(gchunks):
        gsl = slice(gc * 128, (gc + 1) * 128)
        cpp = ps_po.tile([128, 256], F32, tag="cpp")
        for h in range(H):
            for k in range(2):
                nc.tensor.matmul(
                    cpp, sx3[:, k, gsl, h], cw[:, h * 2 + k, :],
                    start=(h == 0 and k == 0),
                    stop=(cb0 and h == 7 and k == 1))
        if not cb0:
            nc.tensor.matmul(cpp, ones1, cbR, start=False, stop=True)
        hpp = ps_po.tile([128, 256], F32, tag="cpp", name="hpp")
        for k in range(2):
            nc.tensor.matmul(
                hpp, sx3[:, k, gsl, 8], hw[:, k, :],
                start=(k == 0), stop=False)
            nc.tensor.matmul(
                hpp, sx3[:, k, gsl, 9], hw[:, 2 + k, :],
                start=False, stop=(hb0 and k == 1))
        if not hb0:
            nc.tensor.matmul(hpp, ones1, hbR, start=False, stop=True)
        csb = posT.tile([128, 256], F32, tag=f"csb{gc}", name=f"csb{gc}")
        nc.vector.tensor_copy(out=csb, in_=cpp)
        hsb = posT.tile([128, 256], F32, tag=f"hsb{gc}", name=f"hsb{gc}")
        nc.scalar.activation(out=hsb, in_=hpp, func=AF.Copy)
        for i, ppx in enumerate((csb, hsb)):
            st6 = pos.tile([128, 6], F32, tag="pst6")
            nc.vector.bn_stats(out=st6, in_=ppx)
            nc.vector.bn_aggr(out=pmv[:, gc * 2 + i, :], in_=st6)
        cps.append((csb, hsb))

    prr = posT.tile([128, 2 * gchunks], F32, tag="prr")
    nc.vector.tensor_scalar(
        out=prr, in0=pmv[:, :, 1], scalar1=1.0, scalar2=1e-5,
        op0=OP.mult, op1=OP.add)
    nc.scalar.activation(out=prr, in_=prr, func=AF.Ln)
    nc.scalar.activation(out=prr, in_=prr, func=AF.Exp, scale=-0.5)

    for gc in range(gchunks):
        gsl = slice(gc * 128, (gc + 1) * 128)
        for i, (ppx, ggB, bbB, g1, bt0) in enumerate((
            (cps[gc][0], cgB, cbtB, cg1, cbt0),
            (cps[gc][1], hgB, hbtB, hg1, hbt0),
        )):
            mcol = pmv[:, gc * 2 + i, 0:1]
            rcol = prr[:, gc * 2 + i:gc * 2 + i + 1]
            rg = pos.tile([128, 256], BF16, tag="prg")
            if g1 and bt0:
                tg = pos.tile([128, 256], F32, tag="ptg")
                nc.vector.tensor_scalar(
                    out=tg, in0=ppx, scalar1=mcol, scalar2=rcol,
                    op0=OP.subtract, op1=OP.mult)
                nc.vector.tensor_scalar(
                    out=rg, in0=tg, scalar1=0.0, scalar2=None, op0=OP.max)
            else:
                tg = pos.tile([128, 256], F32, tag="ptg")
                if g1:
                    nc.vector.tensor_scalar(
                        out=tg, in0=ppx, scalar1=mcol, scalar2=rcol,
                        op0=OP.subtract, op1=OP.mult)
                else:
                    nc.vector.scalar_tensor_tensor(
                        out=tg, in0=ppx, scalar=mcol,
                        in1=ggB, op0=OP.subtract, op1=OP.mult)
                    nc.vector.tensor_scalar(
                        out=tg, in0=tg, scalar1=rcol, scalar2=None,
                        op0=OP.mult)
                if not bt0:
                    nc.vector.tensor_tensor(
                        out=tg, in0=tg, in1=bbB, op=OP.add)
                nc.vector.tensor_scalar(
                    out=rg, in0=tg, scalar1=0.0, scalar2=None, op0=OP.max)
            nc.sync.dma_start_transpose(
                out=catT[:, 2 * i:2 * i + 2, gsl], in_=rg)

    # p1 matmul + LN + relu -> zT
    pmv2 = posT.tile([128, gchunks, 2], F32, tag="pmv2")
    zpps = []
    for gc in range(gchunks):
        gsl = slice(gc * 128, (gc + 1) * 128)
        zpp = ps_pz.tile([128, 512], F32, tag="zpp")
        for kk in range(4):
            nc.tensor.matmul(
                zpp, catT[:, kk, gsl], p1w[:, kk, :],
                start=(kk == 0), stop=False)
        for kk in range(2):
            nc.tensor.matmul(
                zpp, staged_sf[:, kk, gsl], p1w[:, 4 + kk, :],
                start=False, stop=(p1b0 and kk == 1))
        if not p1b0:
            nc.tensor.matmul(zpp, ones1, p1bR, start=False, stop=True)
        zsb = posT.tile([128, 512], F32, tag=f"zsb{gc}", name=f"zsb{gc}")
        nc.vector.tensor_copy(out=zsb, in_=zpp)
        st6 = pos.tile([128, 6], F32, tag="pst6")
        nc.vector.bn_stats(out=st6, in_=zsb)
        nc.vector.bn_aggr(out=pmv2[:, gc, :], in_=st6)
        zpps.append(zsb)

    prr2 = posT.tile([128, gchunks], F32, tag="prr2")
    nc.vector.tensor_scalar(
        out=prr2, in0=pmv2[:, :, 1], scalar1=1.0, scalar2=1e-5,
        op0=OP.mult, op1=OP.add)
    nc.scalar.activation(out=prr2, in_=prr2, func=AF.Ln)
    nc.scalar.activation(out=prr2, in_=prr2, func=AF.Exp, scale=-0.5)

    for gc in range(gchunks):
        gsl = slice(gc * 128, (gc + 1) * 128)
        zsb = zpps[gc]
        mcol = pmv2[:, gc, 0:1]
        rcol = prr2[:, gc:gc + 1]
        rg = pos.tile([128, 512], BF16, tag="prg5")
        if p1g1 and p1bt0:
            tg = pos.tile([128, 512], F32, tag="ptg5")
            nc.vector.tensor_scalar(
                out=tg, in0=zsb, scalar1=mcol, scalar2=rcol,
                op0=OP.subtract, op1=OP.mult)
            nc.vector.tensor_scalar(
                out=rg, in0=tg, scalar1=0.0, scalar2=None, op0=OP.max)
        else:
            tg = pos.tile([128, 512], F32, tag="ptg5")
            if p1g1:
                nc.vector.tensor_scalar(
                    out=tg, in0=zsb, scalar1=mcol, scalar2=rcol,
                    op0=OP.subtract, op1=OP.mult)
            else:
                nc.vector.scalar_tensor_tensor(
                    out=tg, in0=zsb, scalar=mcol,
                    in1=p1gB, op0=OP.subtract, op1=OP.mult)
                nc.vector.tensor_scalar(
                    out=tg, in0=tg, scalar1=rcol, scalar2=None, op0=OP.mult)
            if not p1bt0:
                nc.vector.tensor_tensor(out=tg, in0=tg, in1=p1btB, op=OP.add)
            nc.vector.tensor_scalar(
                out=rg, in0=tg, scalar1=0.0, scalar2=None, op0=OP.max)
        nc.sync.dma_start_transpose(out=zT[:, :, gsl], in_=rg)

    # final projection
    for gc in range(gchunks):
        gsl = slice(gc * 128, (gc + 1) * 128)
        opp = ps_po.tile([128, 256], F32, tag="cpp", name="opp")
        for kk in range(4):
            nc.tensor.matmul(
                opp, zT[:, kk, gsl], p2w[:, kk, :],
                start=(kk == 0), stop=(p2b0 and kk == 3))
        if not p2b0:
            nc.tensor.matmul(opp, ones1, p2bR, start=False, stop=True)
        osb = pos.tile([128, 256], F32, tag="osb")
        nc.scalar.activation(out=osb, in_=opp, func=AF.Copy)
        nc.sync.dma_start(out=out_d.ap()[gsl, :], in_=osb)


# ---------------------------------------------------------------------------
# host side
# ---------------------------------------------------------------------------

_NC_CACHE = {}


def _get_nc(n_mega=FULL_N_MEGA, flags=None, mm_mode=MM_MODE, repeat=1):
    flags = flags or {}
    key = (n_mega, tuple(sorted(flags.items())), mm_mode, SIM_SAFE, repeat)
    if key not in _NC_CACHE:
        _NC_CACHE[key] = build_nc(n_mega, flags, mm_mode, repeat=repeat)
    return _NC_CACHE[key]


def _bf(a):
    return np.ascontiguousarray(np.asarray(a, np.float32).astype(
        ml_dtypes.bfloat16))


def _f8(a):
    return np.ascontiguousarray(np.asarray(a, np.float32).astype(
        ml_dtypes.float8_e4m3))


def _detect_flags(inp):
    f = {}

    def allz(k):
        return bool((np.asarray(inp[k]) == 0).all())

    def all1(k):
        return bool((np.asarray(inp[k]) == 1).all())

    f["spb0"] = allz("sp_b")
    f["spg1"] = all1("sp_g")
    f["spbt0"] = allz("sp_beta")
    f["b20"] = (allz("att_b2") and allz("piece_b2") and allz("empty_b2"))
    f["cb0"] = allz("comb_b")
    f["cg1"] = all1("comb_g")
    f["cbt0"] = allz("comb_beta")
    f["hb0"] = allz("hier_b")
    f["hg1"] = all1("hier_g")
    f["hbt0"] = allz("hier_beta")
    f["p1b0"] = allz("p1_b")
    f["p1g1"] = all1("p1_g")
    f["p1bt0"] = allz("p1_beta")
    f["p2b0"] = allz("p2_b")
    return f


def _prep_weights(inp, mm_mode=MM_MODE):
    f = np.float32
    att_W1 = np.asarray(inp["att_W1"], f)          # [8, 256, 64]
    att_b1 = np.asarray(inp["att_b1"], f)          # [8, 64]
    att_w2 = np.asarray(inp["att_w2"], f)          # [8, 64]
    piece_W1 = np.asarray(inp["piece_W1"], f)      # [256, 128]
    empty_W1 = np.asarray(inp["empty_W1"], f)
    piece_b1 = np.asarray(inp["piece_b1"], f)      # [128]
    empty_b1 = np.asarray(inp["empty_b1"], f)
    piece_w2 = np.asarray(inp["piece_w2"], f)      # [128]
    empty_w2 = np.asarray(inp["empty_w2"], f)

    w1cat = np.transpose(att_W1, (1, 0, 2)).reshape(256, 512)  # [c, h*64+d]
    w1 = w1cat.reshape(2, 128, 512).transpose(1, 0, 2)         # [p, k, col]
    b1a = np.ascontiguousarray(att_b1.reshape(512).reshape(4, 128).T)
    pecat = np.concatenate([piece_W1, empty_W1], 1)            # [256, 256]
    pew = pecat.reshape(2, 128, 256).transpose(1, 0, 2)
    b1p = np.ascontiguousarray(
        np.concatenate([piece_b1, empty_b1]).reshape(2, 128).T)
    w2c = np.zeros((128, 4, 16), f)
    for h in range(H):
        m, half = divmod(h, 2)
        w2c[64 * half:64 * (half + 1), m, h] = att_w2[h]
    pw2c = np.zeros((128, 2, 16), f)
    pw2c[:, 0, 8] = piece_w2
    pw2c[:, 1, 9] = empty_w2
    b2col = np.zeros((80, 1), f)
    att_b2 = np.asarray(inp["att_b2"], f)
    for h in range(8):
        b2col[h * 8:h * 8 + 8, 0] = att_b2[h]
    b2col[64:72, 0] = np.float32(inp["piece_b2"])
    b2col[72:80, 0] = np.float32(inp["empty_b2"])
    spW = np.asarray(inp["sp_W"], np.float64)
    spW = spW - spW.mean(axis=1, keepdims=True)   # exact LN centering
    spwm = spW.astype(f).reshape(2, 128, 256).transpose(1, 0, 2)
    sav = (1.0 / (1.0 + np.exp(-np.asarray(inp["strat_w"], np.float64))))
    sav = np.tile(sav.reshape(64), 2).astype(f).reshape(128, 1)
    c = np.ascontiguousarray
    wd = {
        "w1": _bf(w1), "b1a": c(b1a), "pew": _bf(pew), "b1p": c(b1p),
        "w2c": _bf(w2c), "pw2c": _bf(pw2c), "b2col": c(b2col),
        "spw": _bf(spwm),
        "spb": _bf((np.asarray(inp["sp_b"], np.float64)
                    - np.asarray(inp["sp_b"], np.float64).mean())
                   .astype(f).reshape(1, 256)),
        "spg": c(np.asarray(inp["sp_g"], f).reshape(1, 256)),
        "spbt": c(np.asarray(inp["sp_beta"], f).reshape(1, 256)),
        "sa": c(sav), "sa2": c(sav * sav),
        "cw": _bf(np.asarray(inp["comb_W"], f).reshape(16, 128, 256)
                  .transpose(1, 0, 2)),
        "cb": _bf(np.asarray(inp["comb_b"], f).reshape(1, 256)),
        "cg": c(np.asarray(inp["comb_g"], f).reshape(1, 256)),
        "cbt": c(np.asarray(inp["comb_beta"], f).reshape(1, 256)),
        "hw": _bf(np.asarray(inp["hier_W"], f).reshape(4, 128, 256)
                  .transpose(1, 0, 2)),
        "hb": _bf(np.asarray(inp["hier_b"], f).reshape(1, 256)),
        "hg": c(np.asarray(inp["hier_g"], f).reshape(1, 256)),
        "hbt": c(np.asarray(inp["hier_beta"], f).reshape(1, 256)),
        "p1w": _bf(np.asarray(inp["p1_W"], f).reshape(6, 128, 512)
                   .transpose(1, 0, 2)),
        "p1b": _bf(np.asarray(inp["p1_b"], f).reshape(1, 512)),
        "p1g": c(np.asarray(inp["p1_g"], f).reshape(1, 512)),
        "p1bt": c(np.asarray(inp["p1_beta"], f).reshape(1, 512)),
        "p2w": _bf(np.asarray(inp["p2_W"], f).reshape(4, 128, 256)
                   .transpose(1, 0, 2)),
        "p2b": _bf(np.asarray(inp["p2_b"], f).reshape(1, 256)),
    }
    if mm_mode == "fp8":
        wd["w1q"] = _f8(w1)
        wd["pewq"] = _f8(pew)
        wd["spwq"] = _f8(spwm)
        # att L2 stationary in DR pairs: [p, i(k of hL pair), mpair, col]
        w2cq = np.zeros((128, 2, 2, 16), f)
        w2cq[:, 0, 0, :] = w2c[:, 0, :]
        w2cq[:, 1, 0, :] = w2c[:, 1, :]
        w2cq[:, 0, 1, :] = w2c[:, 2, :]
        w2cq[:, 1, 1, :] = w2c[:, 3, :]
        wd["w2cq"] = _f8(w2cq)
        wd["pw2cq"] = _f8(pw2c)
    return wd


def make_in_maps(inputs, n_mega=FULL_N_MEGA, mm_mode=MM_MODE):
    x = np.asarray(inputs["x"], np.float32)
    nt = np.asarray(inputs["node_types"])
    mb = (nt > 0).astype(ml_dtypes.bfloat16)
    nb = (nt <= 0).astype(ml_dtypes.bfloat16)
    xb = x.astype(ml_dtypes.bfloat16)
    wd = _prep_weights(inputs, mm_mode)
    nodes_pc = n_mega * MEGA * ST
    in_maps = []
    for cc in range(N_CORES):
        sl = slice(cc * nodes_pc, (cc + 1) * nodes_pc)
        m = {"x": np.ascontiguousarray(xb[sl]),
             "mb": np.ascontiguousarray(mb[sl]),
             "nb": np.ascontiguousarray(nb[sl])}
        m.update(wd)
        in_maps.append(m)
    return in_maps


def run(inputs, n_mega=FULL_N_MEGA, mm_mode=MM_MODE):
    flags = _detect_flags(inputs)
    nc = _get_nc(n_mega, flags, mm_mode)
    in_maps = make_in_maps(inputs, n_mega, mm_mode)
    res = run_bass_kernel_spmd(nc, in_maps, core_ids=list(range(N_CORES)))
    return np.concatenate(
        [res.results[cc]["out"] for cc in range(N_CORES)], axis=0
    )


def kernel(**inputs):
    return run(inputs, FULL_N_MEGA)


# revision 26
# speedup vs baseline: 7.5408x; 6.0911x over previous
"""Trainium2 Bass kernel for nn_ChessGraphPooling (segment_reduce).

Data-parallel over whole graphs: 4096 boards x 64 nodes sharded across 8
NeuronCores (512 graphs / 32768 nodes per core); small weights replicated.

v3 design (bf16 pipeline, engine-balanced):
  - x converted to bf16 on host; per-core DMA traffic halves.
  - node-layout x tiles feed pooling matmuls as stationary operands and are
    transposed into channel-major T-layout [c, nodes] by the DMA engines'
    XBAR transpose (zero PE/DVE cost).
  - scorer linears run on the PE in T-layout; hidden activations (LeakyRelu)
    on the scalar engine; tiny second layers are [10,512] matmuls.
  - segment softmax skips the max-subtract pass (exact: softmax is
    shift-invariant and scores are O(1), exp cannot overflow); scores are
    stacked per-mega (8 supertiles, 80 rows) so every softmax op is one
    free-size-bound pass.
  - all rsqrt = Exp(-0.5*Ln(x)) so the ACT engine keeps one activation
    table (Prelu/Exp/Ln/Square live in natural_log_exp_and_others) and never
    pays the 1.3us table reload.
  - strategic branch LN runs in node layout via bn_stats; normalize+relu is
    a single fused tensor_scalar (sub,mult / mult,max) when gamma==1/beta==0
    (detected on host, the general path adds the row ops back).
  - pooling writes T-layout directly: stationary = x/sf node-layout chunks,
    moving = per-graph weight columns; pooled results land in PSUM already
    channel-major, so no pool transposes and no re-staging.
  - post stage (per-graph MLPs) reuses the same tricks: bf16 matmuls, DMA
    transposes, Pool-engine PSUM drains.
"""

import os
import sys

sys.path.insert(0, "/opt/trn_rl_repo")

from contextlib import ExitStack

import numpy as np
import ml_dtypes

import concourse.bass as bass
import concourse.bacc as bacc
import concourse.tile as tile
import concourse.mybir as mybir
from concourse.bass_utils import run_bass_kernel_spmd

F32 = mybir.dt.float32
BF16 = mybir.dt.bfloat16
FP8 = mybir.dt.float8e4
I32 = mybir.dt.int32
AF = mybir.ActivationFunctionType
OP = mybir.AluOpType
AX = mybir.AxisListType
DRMODE = mybir.MatmulPerfMode.DoubleRow

C = 256
H = 8
NODES = 64
NEG = 0.2
N_CORES = 8
ST = 512          # nodes per supertile
CHUNKS = 4        # 128-node chunks per supertile
MEGA = 8          # supertiles per megatile (80 score rows)
FULL_N_MEGA = 8   # megatiles per core at full size

# matmul input mode: "bf16" or "fp8" (DoubleRow) for the scorer/strat linears
MM_MODE = os.environ.get("K_MM_MODE", "fp8")
# CoreSim does not implement Prelu; sim_safe swaps it for an Abs-based
# decomposition (0.6*v + 0.4*|v|) with identical numerics
SIM_SAFE = bool(os.environ.get("K_SIM_SAFE"))


def build_nc(n_mega=FULL_N_MEGA, flags=None, mm_mode=MM_MODE,
             sim_safe=None, repeat=1):
    flags = dict(flags or {})
    flags["_sim_safe"] = SIM_SAFE if sim_safe is None else sim_safe
    nodes_pc = n_mega * MEGA * ST
    graphs_pc = nodes_pc // NODES
    assert graphs_pc % 128 == 0

    nc = bacc.Bacc("TRN2", num_devices=N_CORES)

    dt = {}

    def din(name, shape, dtype=F32):
        dt[name] = nc.dram_tensor(name, shape, dtype, kind="ExternalInput")

    din("x", [nodes_pc, C], BF16)
    din("mb", [nodes_pc], BF16)       # piece mask (node_types > 0)
    din("nb", [nodes_pc], BF16)       # 1 - piece mask
    din("w1", [128, 2, 512], BF16)    # att W1, [p, k, h*64+d]
    din("b1a", [128, 4])              # att b1 cols per m-chunk
    din("pew", [128, 2, 256], BF16)   # piece|empty W1
    din("b1p", [128, 2])
    din("w2c", [128, 4, 16], BF16)    # att w2 -> score rows 0..7
    din("pw2c", [128, 2, 16], BF16)   # piece/empty w2 -> rows 8,9
    din("b2col", [80, 1])
    din("spw", [128, 2, 256], BF16)
    din("spb", [1, 256], BF16)
    din("spg", [1, 256])
    din("spbt", [1, 256])
    din("sa", [128, 1])
    din("sa2", [128, 1])
    din("cw", [128, 16, 256], BF16)
    din("cb", [1, 256], BF16)
    din("cg", [1, 256])
    din("cbt", [1, 256])
    din("hw", [128, 4, 256], BF16)
    din("hb", [1, 256], BF16)
    din("hg", [1, 256])
    din("hbt", [1, 256])
    din("p1w", [128, 6, 512], BF16)
    din("p1b", [1, 512], BF16)
    din("p1g", [1, 512])
    din("p1bt", [1, 512])
    din("p2w", [128, 4, 256], BF16)
    din("p2b", [1, 256], BF16)
    if mm_mode == "fp8":
        din("w1q", [128, 2, 512], FP8)
        din("pewq", [128, 2, 256], FP8)
        din("spwq", [128, 2, 256], FP8)
        din("w2cq", [128, 2, 2, 16], FP8)   # [p, i, mpair, col]
        din("pw2cq", [128, 2, 16], FP8)     # pe pair: [p, i(m), col]
    out_d = nc.dram_tensor("out", [graphs_pc, C], F32, kind="ExternalOutput")

    with tile.TileContext(nc) as tc:
        for _rep in range(repeat):
            _build_body(nc, tc, n_mega, graphs_pc, dt, out_d, flags, mm_mode)
    nc.compile()
    return nc


def _bcast(nc, dst, src_d):
    nc.gpsimd.dma_start(
        out=dst, in_=src_d.ap().partition_broadcast(dst.shape[0])
    )


def _build_body(nc, tc, n_mega, graphs_pc, dt, out_d, flags, mm_mode):
    gchunks = graphs_pc // 128
    fp8 = mm_mode == "fp8"

    nc.scalar.add_instruction(mybir.InstLoadActFuncSet(
        name=nc.get_next_instruction_name(), ins=[], outs=[],
        act_func_set_id=6))  # natural_log_exp_and_others: Prelu+Exp+Ln

    with ExitStack() as top:
        consts = top.enter_context(tc.tile_pool(name="consts", bufs=1))
        persist = top.enter_context(tc.tile_pool(name="persist", bufs=1))

        def cload(name, shape, dtype=F32, src=None):
            t = consts.tile(shape, dtype, tag=name, name=name)
            nc.sync.dma_start(out=t, in_=dt[src or name].ap())
            return t

        w1 = cload("w1", [128, 2, 512], BF16)
        b1a = cload("b1a", [128, 4])
        pew = cload("pew", [128, 2, 256], BF16)
        b1p = cload("b1p", [128, 2])
        w2c = cload("w2c", [128, 4, 16], BF16)
        pw2c = cload("pw2c", [128, 2, 16], BF16)
        b2col = cload("b2col", [80, 1])
        spw = cload("spw", [128, 2, 256], BF16)
        sa = cload("sa", [128, 1])
        sa2 = cload("sa2", [128, 1])
        if fp8:
            w1q = cload("w1q", [128, 2, 512], FP8)
            pewq = cload("pewq", [128, 2, 256], FP8)
            spwq = cload("spwq", [128, 2, 256], FP8)
            w2cq = cload("w2cq", [128, 2, 2, 16], FP8)
            pw2cq = cload("pw2cq", [128, 2, 16], FP8)

        ones1 = consts.tile([1, 128], BF16, tag="ones1")
        nc.vector.memset(ones1, 1.0)
        spbrow = None
        if not flags.get("spb0", False):
            spbrow = cload("spbrow", [1, 256], BF16, src="spb")
        spgB = spbtB = None
        if not flags.get("spg1", False):
            spgB = consts.tile([128, 256], F32, tag="spgB")
            _bcast(nc, spgB, dt["spg"])
        if not flags.get("spbt0", False):
            spbtB = consts.tile([128, 256], F32, tag="spbtB")
            _bcast(nc, spbtB, dt["spbt"])

        poolcol = consts.tile([128, 2], BF16, tag="poolcol")
        nc.vector.memset(poolcol, 0.0)
        nc.vector.memset(poolcol[0:64, 0:1], 1.0 / NODES)
        nc.vector.memset(poolcol[64:128, 1:2], 1.0 / NODES)
        blockmask = consts.tile([128, 2, 10], BF16, tag="blockmask")
        nc.vector.memset(blockmask, 0.0)
        nc.vector.memset(blockmask[0:64, 0, :], 1.0)
        nc.vector.memset(blockmask[64:128, 1, :], 1.0)
        # double-buffered mask stacks; att rows stay 1.0 forever
        maskS = []
        for i in range(2):
            mt = consts.tile([80, 512], BF16, tag=f"maskS{i}", name=f"maskS{i}")
            nc.vector.memset(mt, 1.0)
            maskS.append(mt)

        staged_x = persist.tile(
            [128, 2, graphs_pc * 10], BF16, tag="staged_x")
        staged_sf = persist.tile([128, 2, graphs_pc], BF16, tag="staged_sf")

        with ExitStack() as main:
            xpool = main.enter_context(tc.tile_pool(name="xpool", bufs=9))
            xTp = main.enter_context(tc.tile_pool(name="xTp", bufs=5))
            actp = main.enter_context(tc.tile_pool(name="actp", bufs=6))
            sfp = main.enter_context(tc.tile_pool(name="sfp", bufs=4))
            wcp = main.enter_context(tc.tile_pool(name="wcp", bufs=4))
            scr = main.enter_context(tc.tile_pool(name="scr", bufs=8))
            megap = main.enter_context(tc.tile_pool(name="megap", bufs=3))

            ps_mm = main.enter_context(
                tc.tile_pool(name="ps_mm", bufs=3, space="PSUM"))
            ps_sc = main.enter_context(
                tc.tile_pool(name="ps_sc", bufs=2, space="PSUM"))
            ps_pool = main.enter_context(
                tc.tile_pool(name="ps_pool", bufs=2, space="PSUM"))
            ps_sfp = main.enter_context(
                tc.tile_pool(name="ps_sfp", bufs=1, space="PSUM"))

            for mg in range(n_mega):
                _mega_body(
                    nc, tc, mg, dt, staged_x, staged_sf,
                    w1q if fp8 else w1, b1a, pewq if fp8 else pew, b1p,
                    w2cq if fp8 else w2c, pw2cq if fp8 else pw2c, b2col,
                    spwq if fp8 else spw, spbrow, spgB, spbtB, sa, sa2,
                    ones1, poolcol, blockmask, maskS[mg % 2],
                    xpool, xTp, actp, sfp, wcp, scr, megap,
                    ps_mm, ps_sc, ps_pool, ps_sfp, flags, fp8,
                )

        with ExitStack() as post:
            posw = post.enter_context(tc.tile_pool(name="posw", bufs=1))
            pos = post.enter_context(tc.tile_pool(name="pos", bufs=4))
            posT = post.enter_context(tc.tile_pool(name="posT", bufs=1))
            ps_po = post.enter_context(
                tc.tile_pool(name="ps_po", bufs=2, space="PSUM"))
            ps_pz = post.enter_context(
                tc.tile_pool(name="ps_pz", bufs=2, space="PSUM"))
            _post_body(
                nc, tc, graphs_pc, gchunks, dt, staged_x, staged_sf,
                ones1, posw, pos, posT, ps_po, ps_pz, out_d, flags,
            )


def _prelu(nc, scr, out, ph, bias_col, sim_safe):
    """out = LeakyRelu(ph + bias, NEG)."""
    if not sim_safe:
        nc.scalar.activation(
            out=out, in_=ph, func=AF.Prelu,
            bias=bias_col, scale=1.0, alpha=NEG)
        return
    shape = [ph.shape[0], ph.shape[-1]]
    absv = scr.tile(shape, F32, tag="pabs", name="pabs")
    nc.scalar.activation(
        out=absv, in_=ph, func=AF.Abs, bias=bias_col, scale=1.0)
    t = scr.tile(shape, F32, tag="pt", name="pt")
    nc.vector.tensor_scalar(
        out=t, in0=ph, scalar1=bias_col, scalar2=(1.0 + NEG) / 2.0,
        op0=OP.add, op1=OP.mult)
    nc.vector.scalar_tensor_tensor(
        out=out, in0=absv, scalar=(1.0 - NEG) / 2.0,
        in1=t, op0=OP.mult, op1=OP.add)


def _mega_body(
    nc, tc, mg, dt, staged_x, staged_sf,
    w1, b1a, pew, b1p, w2c, pw2c, b2col, spw, spbrow, spgB, spbtB, sa, sa2,
    ones1, poolcol, blockmask, maskS,
    xpool, xTp, actp, sfp, wcp, scr, megap,
    ps_mm, ps_sc, ps_pool, ps_sfp, flags, fp8,
):
    sim_safe = flags.get("_sim_safe", False)
    spb0 = flags.get("spb0", False)
    spg1 = flags.get("spg1", False)
    spbt0 = flags.get("spbt0", False)
    b20 = flags.get("b20", False)
    mmdt = FP8 if fp8 else BF16

    # per-mega pe-score masks: blocked rows (att 0:64, piece 64:72, empty 72:80)
    nc.sync.dma_start(
        out=maskS[64:72, :],
        in_=dt["mb"].ap()[mg * MEGA * ST:(mg + 1) * MEGA * ST]
        .rearrange("(s n) -> s n", s=8),
    )
    nc.sync.dma_start(
        out=maskS[72:80, :],
        in_=dt["nb"].ap()[mg * MEGA * ST:(mg + 1) * MEGA * ST]
        .rearrange("(s n) -> s n", s=8),
    )

    sstack = megap.tile([80, 512], BF16, tag="sstack")
    stbuf = megap.tile([16, 8, 512], BF16, tag="stbuf")
    xs = []
    xsb2 = None
    for s8 in range(MEGA):
        s = mg * MEGA + s8
        if s8 % 2 == 0:
            xsb2 = xpool.tile([128, 8, 256], BF16, tag="xsb")
            nc.sync.dma_start(
                out=xsb2,
                in_=dt["x"].ap()[s * ST:(s + 2) * ST, :]
                .rearrange("(c p) m -> p c m", p=128),
            )
        xsb = xsb2[:, (s8 % 2) * 4:(s8 % 2) * 4 + 4, :]
        xs.append(xsb)

        # one XBAR transpose per supertile over the flat [128,1024] view:
        # xTf[p, 2c+k, n] = x[node c*128+n, k*128+p]
        xTf = xTp.tile([128, 8, 128], BF16, tag="xT")
        nc.sync.dma_start_transpose(
            out=xTf, in_=xsb.rearrange("p c m -> p (c m)"))
        if fp8:
            # shuffle-convert to standard DR layout [p, k, c*128+n]
            xTq = xTp.tile([128, 2, 512], FP8, tag="xTq", name="xTq")
            eng = nc.gpsimd
            eng.tensor_copy(
                out=xTq.rearrange("p k (c n) -> p k c n", n=128),
                in_=xTf.rearrange("p (c k) n -> p k c n", k=2))
            xmv = xTq
        else:
            xmv = xTf.rearrange("p (c k) n -> p k c n", k=2)

        scp = ps_sc.tile([16, 512], F32, tag="scp")
        # attention scorer
        hLt = None
        for m in range(4):
            ph = ps_mm.tile([128, 512], F32, tag="ph")
            if fp8:
                nc.tensor.matmul(
                    ph, w1[:, :, m * 128:(m + 1) * 128], xmv,
                    start=True, stop=True, perf_mode=DRMODE)
            else:
                nc.tensor.matmul(
                    ph, w1[:, 0, m * 128:(m + 1) * 128], xmv[:, 0],
                    start=True, stop=False)
                nc.tensor.matmul(
                    ph, w1[:, 1, m * 128:(m + 1) * 128], xmv[:, 1],
                    start=False, stop=True)
            if m % 2 == 0:
                hLt = actp.tile([128, 2, 512], mmdt, tag="hL")
            _prelu(nc, scr, hLt[:, m % 2, :], ph, b1a[:, m:m + 1], sim_safe)
            if fp8:
                if m % 2 == 1:
                    nc.tensor.matmul(
                        scp[0:10, :], w2c[:, :, m // 2, 0:10], hLt,
                        start=(m == 1), stop=False, perf_mode=DRMODE)
            else:
                nc.tensor.matmul(
                    scp[0:10, :], w2c[:, m, 0:10], hLt[:, m % 2, :],
                    start=(m == 0), stop=False)

        # piece/empty scorer
        peLt = actp.tile([128, 2, 512], mmdt, tag="peL", name="peL")
        for m in range(2):
            pp = ps_mm.tile([128, 512], F32, tag="ph", name="pp")
            if fp8:
                nc.tensor.matmul(
                    pp, pew[:, :, m * 128:(m + 1) * 128], xmv,
                    start=True, stop=True, perf_mode=DRMODE)
            else:
                nc.tensor.matmul(
                    pp, pew[:, 0, m * 128:(m + 1) * 128], xmv[:, 0],
                    start=True, stop=False)
                nc.tensor.matmul(
                    pp, pew[:, 1, m * 128:(m + 1) * 128], xmv[:, 1],
                    start=False, stop=True)
            _prelu(nc, scr, peLt[:, m, :], pp, b1p[:, m:m + 1], sim_safe)
        if fp8:
            nc.tensor.matmul(
                scp[0:10, :], pw2c[:, :, 0:10], peLt,
                start=False, stop=True, perf_mode=DRMODE)
        else:
            for m in range(2):
                nc.tensor.matmul(
                    scp[0:10, :], pw2c[:, m, 0:10], peLt[:, m, :],
                    start=False, stop=(m == 1))

        # drain scores into the SBUF stage buffer (no DMA)
        nc.vector.tensor_copy(out=stbuf[0:10, s8, :], in_=scp[0:10, :])

        # strategic branch: z' = x @ (spW - rowmean(spW)) is exactly
        # centered, so LN needs only the variance; rho is computed
        # per-supertile and the PSUM drain fuses scale+relu in one op.
        mv4 = scr.tile([128, 4, 2], F32, tag="mv4", name="mv4")
        pzs = []
        for cp in range(2):
            pz = ps_mm.tile([128, 512], F32, tag="ph", name="pz")
            pzs.append(pz)
            for half in range(2):
                sec = cp * 2 + half
                sl = pz[:, half * 256:(half + 1) * 256]
                last = spb0
                if fp8:
                    nc.tensor.matmul(
                        sl, xmv[:, :, sec * 128:(sec + 1) * 128], spw,
                        start=True, stop=last, perf_mode=DRMODE)
                else:
                    nc.tensor.matmul(
                        sl, xmv[:, 0, sec], spw[:, 0, :],
                        start=True, stop=False)
                    nc.tensor.matmul(
                        sl, xmv[:, 1, sec], spw[:, 1, :],
                        start=False, stop=last)
                if not spb0:
                    nc.tensor.matmul(sl, ones1, spbrow, start=False, stop=True)
            for half in range(2):
                sec = cp * 2 + half
                sl = pz[:, half * 256:(half + 1) * 256]
                st6 = scr.tile([128, 6], F32, tag="st6")
                nc.vector.bn_stats(out=st6, in_=sl)
                nc.vector.bn_aggr(out=mv4[:, sec, :], in_=st6)
        rho4 = scr.tile([128, 4], F32, tag="rho4", name="rho4")
        nc.vector.tensor_scalar(
            out=rho4, in0=mv4[:, :, 1], scalar1=sa2, scalar2=1e-5,
            op0=OP.mult, op1=OP.add)
        nc.scalar.activation(out=rho4, in_=rho4, func=AF.Ln)
        nc.scalar.activation(out=rho4, in_=rho4, func=AF.Exp, scale=-0.5)
        nc.vector.tensor_scalar(
            out=rho4, in0=rho4, scalar1=sa, scalar2=None, op0=OP.mult)
        sff = sfp.tile([128, 4, 256], BF16, tag="sf")
        for cp in range(2):
            pz = pzs[cp]
            for half in range(2):
                sec = cp * 2 + half
                sl = pz[:, half * 256:(half + 1) * 256]
                rcol = rho4[:, sec:sec + 1]
                if spg1 and spbt0:
                    if sec != 0:
                        nc.vector.tensor_scalar(
                            out=sff[:, sec, :], in0=sl, scalar1=rcol,
                            scalar2=0.0, op0=OP.mult, op1=OP.max)
                    else:
                        nc.scalar.activation(
                            out=sff[:, sec, :], in_=sl, func=AF.Relu,
                            scale=rcol, bias=0.0)
                else:
                    tg = scr.tile([128, 256], F32, tag="tg", name="tg")
                    nc.vector.tensor_scalar(
                        out=tg, in0=sl, scalar1=rcol, scalar2=None,
                        op0=OP.mult)
                    if spgB is not None:
                        nc.vector.tensor_tensor(
                            out=tg, in0=tg, in1=spgB, op=OP.mult)
                    if spbtB is not None:
                        nc.vector.tensor_tensor(
                            out=tg, in0=tg, in1=spbtB, op=OP.add)
                    nc.vector.tensor_scalar(
                        out=sff[:, sec, :], in0=tg, scalar1=0.0,
                        scalar2=None, op0=OP.max)
        # sf mean-pool straight to T-layout and stage it (phase A)
        psf = ps_sfp.tile([128, 2, 8], F32, tag="psf")
        for sec in range(CHUNKS):
            for k in range(2):
                nc.tensor.matmul(
                    psf[:, k, sec * 2:sec * 2 + 2],
                    sff[:, sec, k * 128:(k + 1) * 128], poolcol,
                    start=True, stop=True)
        nc.vector.tensor_copy(
            out=staged_sf[:, :, s * 8:(s + 1) * 8], in_=psf)

    # assemble the mega score stack: 3 DMAs
    nc.sync.dma_start(out=sstack[0:64, :], in_=stbuf[0:8])
    nc.sync.dma_start(out=sstack[64:72, :], in_=stbuf[8:9])
    nc.sync.dma_start(out=sstack[72:80, :], in_=stbuf[9:10])

    # ---- phase B: batched segment softmax (no max-subtract; exact) ----
    if b20:
        nc.gpsimd.tensor_tensor(
            out=sstack, in0=sstack, in1=maskS, op=OP.mult)
    else:
        nc.gpsimd.scalar_tensor_tensor(
            out=sstack, in0=sstack, scalar=b2col,
            in1=maskS, op0=OP.add, op1=OP.mult)
    estack = megap.tile([80, 512], BF16, tag="estack")
    nc.scalar.activation(out=estack, in_=sstack, func=AF.Exp)
    dsum = megap.tile([80, 8], F32, tag="dsum")
    nc.vector.tensor_reduce(
        out=dsum, in_=estack.rearrange("p (g n) -> p g n", n=NODES),
        axis=AX.X, op=OP.add)
    nc.vector.tensor_scalar(
        out=dsum, in0=dsum, scalar1=1e-16, scalar2=None, op0=OP.add)
    dre = megap.tile([80, 8], BF16, tag="dre")
    with nc.allow_low_precision(reason="softmax denom reciprocal, bf16 ok"):
        nc.vector.reciprocal(out=dre, in_=dsum)
    wT = megap.tile([80, 512], BF16, tag="wT")
    nc.gpsimd.tensor_tensor(
        out=wT.rearrange("p (g n) -> p g n", n=NODES),
        in0=estack.rearrange("p (g n) -> p g n", n=NODES),
        in1=dre.unsqueeze(2).broadcast_to([80, 8, NODES]),
        op=OP.mult)
    # wtt[p, c, r] = wT[r, c*128+p]
    wtt = megap.tile([128, 4, 80], BF16, tag="wtt")
    nc.sync.dma_start_transpose(out=wtt, in_=wT)

    # ---- phase C: attention/piece/empty pooling ----
    for s8 in range(MEGA):
        s = mg * MEGA + s8
        xsb = xs[s8]
        pooled = ps_pool.tile([128, 2, 80], F32, tag="pooled")
        wc4 = wcp.tile([128, 4, 2, 10], BF16, tag="wc4")
        nc.gpsimd.tensor_tensor(
            out=wc4[:, :, :, 0:8],
            in0=wtt[:, :, s8:64:8].unsqueeze(2).broadcast_to([128, 4, 2, 8]),
            in1=blockmask[:, :, 0:8].unsqueeze(1).broadcast_to([128, 4, 2, 8]),
            op=OP.mult)
        nc.gpsimd.tensor_tensor(
            out=wc4[:, :, :, 8:9],
            in0=wtt[:, :, 64 + s8:65 + s8]
            .unsqueeze(2).broadcast_to([128, 4, 2, 1]),
            in1=blockmask[:, :, 8:9].unsqueeze(1).broadcast_to([128, 4, 2, 1]),
            op=OP.mult)
        nc.gpsimd.tensor_tensor(
            out=wc4[:, :, :, 9:10],
            in0=wtt[:, :, 72 + s8:73 + s8]
            .unsqueeze(2).broadcast_to([128, 4, 2, 1]),
            in1=blockmask[:, :, 9:10].unsqueeze(1).broadcast_to([128, 4, 2, 1]),
            op=OP.mult)
        for sec in range(CHUNKS):
            wcf = wc4[:, sec].rearrange("p a b -> p (a b)")
            for k in range(2):
                nc.tensor.matmul(
                    pooled[:, k, sec * 20:sec * 20 + 20],
                    xsb[:, sec, k * 128:(k + 1) * 128], wcf,
                    start=True, stop=True)
        nc.vector.tensor_copy(
            out=staged_x[:, :, s * 80:(s + 1) * 80], in_=pooled)


def _post_body(
    nc, tc, graphs_pc, gchunks, dt, staged_x, staged_sf,
    ones1, posw, pos, posT, ps_po, ps_pz, out_d, flags,
):
    cb0 = flags.get("cb0", False)
    cg1 = flags.get("cg1", False)
    cbt0 = flags.get("cbt0", False)
    hb0 = flags.get("hb0", False)
    hg1 = flags.get("hg1", False)
    hbt0 = flags.get("hbt0", False)
    p1b0 = flags.get("p1b0", False)
    p1g1 = flags.get("p1g1", False)
    p1bt0 = flags.get("p1bt0", False)
    p2b0 = flags.get("p2b0", False)

    def pload(name, shape, dtype=BF16):
        t = posw.tile(shape, dtype, tag=name, name=name)
        nc.sync.dma_start(out=t, in_=dt[name].ap())
        return t

    cw = pload("cw", [128, 16, 256])
    hw = pload("hw", [128, 4, 256])
    p1w = pload("p1w", [128, 6, 512])
    p2w = pload("p2w", [128, 4, 256])
    cbR = None if cb0 else pload("cb", [1, 256])
    hbR = None if hb0 else pload("hb", [1, 256])
    p1bR = None if p1b0 else pload("p1b", [1, 512])
    p2bR = None if p2b0 else pload("p2b", [1, 256])

    def bc(name, cols, skip):
        if skip:
            return None
        t = posw.tile([128, cols], F32, tag=f"{name}B", name=f"{name}B")
        _bcast(nc, t, dt[name])
        return t

    cgB = bc("cg", 256, cg1)
    cbtB = bc("cbt", 256, cbt0)
    hgB = bc("hg", 256, hg1)
    hbtB = bc("hbt", 256, hbt0)
    p1gB = bc("p1g", 512, p1g1)
    p1btB = bc("p1bt", 512, p1bt0)

    sx3 = staged_x.rearrange("p k (g t) -> p k g t", t=10)

    catT = posT.tile([128, 4, graphs_pc], BF16, tag="catT")
    zT = posT.tile([128, 4, graphs_pc], BF16, tag="zT")
    pmv = posT.tile([128, 2 * gchunks, 2], F32, tag="pmv")

    cps = []
    for gc in range(gchunks):
        gsl = slice(gc * 128, (gc + 1) * 128)
        cpp = ps_po.tile([128, 256], F32, tag="cpp")
        for h in range(H):
            for k in range(2):
                nc.tensor.matmul(
                    cpp, sx3[:, k, gsl, h], cw[:, h * 2 + k, :],
                    start=(h == 0 and k == 0),
                    stop=(cb0 and h == 7 and k == 1))
        if not cb0:
            nc.tensor.matmul(cpp, ones1, cbR, start=False, stop=True)
        hpp = ps_po.tile([128, 256], F32, tag="cpp", name="hpp")
        for k in range(2):
            nc.tensor.matmul(
                hpp, sx3[:, k, gsl, 8], hw[:, k, :],
                start=(k == 0), stop=False)
            nc.tensor.matmul(
                hpp, sx3[:, k, gsl, 9], hw[:, 2 + k, :],
                start=False, stop=(hb0 and k == 1))
        if not hb0:
            nc.tensor.matmul(hpp, ones1, hbR, start=False, stop=True)
        csb = posT.tile([128, 256], F32, tag=f"csb{gc}", name=f"csb{gc}")
        nc.vector.tensor_copy(out=csb, in_=cpp)
        hsb = posT.tile([128, 256], F32, tag=f"hsb{gc}", name=f"hsb{gc}")
        nc.scalar.activation(out=hsb, in_=hpp, func=AF.Copy)
        for i, ppx in enumerate((csb, hsb)):
            st6 = pos.tile([128, 6], F32, tag="pst6")
            nc.vector.bn_stats(out=st6, in_=ppx)
            nc.vector.bn_aggr(out=pmv[:, gc * 2 + i, :], in_=st6)
        cps.append((csb, hsb))

    prr = posT.tile([128, 2 * gchunks], F32, tag="prr")
    nc.vector.tensor_scalar(
        out=prr, in0=pmv[:, :, 1], scalar1=1.0, scalar2=1e-5,
        op0=OP.mult, op1=OP.add)
    nc.scalar.activation(out=prr, in_=prr, func=AF.Ln)
    nc.scalar.activation(out=prr, in_=prr, func=AF.Exp, scale=-0.5)

    for gc in range(gchunks):
        gsl = slice(gc * 128, (gc + 1) * 128)
        for i, (ppx, ggB, bbB, g1, bt0) in enumerate((
            (cps[gc][0], cgB, cbtB, cg1, cbt0),
            (cps[gc][1], hgB, hbtB, hg1, hbt0),
        )):
            mcol = pmv[:, gc * 2 + i, 0:1]
            rcol = prr[:, gc * 2 + i:gc * 2 + i + 1]
            rg = pos.tile([128, 256], BF16, tag="prg")
            if g1 and bt0:
                tg = pos.tile([128, 256], F32, tag="ptg")
                nc.vector.tensor_scalar(
                    out=tg, in0=ppx, scalar1=mcol, scalar2=rcol,
                    op0=OP.subtract, op1=OP.mult)
                nc.vector.tensor_scalar(
                    out=rg, in0=tg, scalar1=0.0, scalar2=None, op0=OP.max)
            else:
                tg = pos.tile([128, 256], F32, tag="ptg")
                if g1:
                    nc.vector.tensor_scalar(
                        out=tg, in0=ppx, scalar1=mcol, scalar2=rcol,
                        op0=OP.subtract, op1=OP.mult)
                else:
                    nc.vector.scalar_tensor_tensor(
                        out=tg, in0=ppx, scalar=mcol,
                        in1=ggB, op0=OP.subtract, op1=OP.mult)
                    nc.vector.tensor_scalar(
                        out=tg, in0=tg, scalar1=rcol, scalar2=None,
                        op0=OP.mult)
                if not bt0:
                    nc.vector.tensor_tensor(
                        out=tg, in0=tg, in1=bbB, op=OP.add)
                nc.vector.tensor_scalar(
                    out=rg, in0=tg, scalar1=0.0, scalar2=None, op0=OP.max)
            nc.sync.dma_start_transpose(
                out=catT[:, 2 * i:2 * i + 2, gsl], in_=rg)

    # p1 matmul + LN + relu -> zT
    pmv2 = posT.tile([128, gchunks, 2], F32, tag="pmv2")
    zpps = []
    for gc in range(gchunks):
        gsl = slice(gc * 128, (gc + 1) * 128)
        zpp = ps_pz.tile([128, 512], F32, tag="zpp")
        for kk in range(4):
            nc.tensor.matmul(
                zpp, catT[:, kk, gsl], p1w[:, kk, :],
                start=(kk == 0), stop=False)
        for kk in range(2):
            nc.tensor.matmul(
                zpp, staged_sf[:, kk, gsl], p1w[:, 4 + kk, :],
                start=False, stop=(p1b0 and kk == 1))
        if not p1b0:
            nc.tensor.matmul(zpp, ones1, p1bR, start=False, stop=True)
        zsb = posT.tile([128, 512], F32, tag=f"zsb{gc}", name=f"zsb{gc}")
        nc.vector.tensor_copy(out=zsb, in_=zpp)
        st6 = pos.tile([128, 6], F32, tag="pst6")
        nc.vector.bn_stats(out=st6, in_=zsb)
        nc.vector.bn_aggr(out=pmv2[:, gc, :], in_=st6)
        zpps.append(zsb)

    prr2 = posT.tile([128, gchunks], F32, tag="prr2")
    nc.vector.tensor_scalar(
        out=prr2, in0=pmv2[:, :, 1], scalar1=1.0, scalar2=1e-5,
        op0=OP.mult, op1=OP.add)
    nc.scalar.activation(out=prr2, in_=prr2, func=AF.Ln)
    nc.scalar.activation(out=prr2, in_=prr2, func=AF.Exp, scale=-0.5)

    for gc in range(gchunks):
        gsl = slice(gc * 128, (gc + 1) * 128)
        zsb = zpps[gc]
        mcol = pmv2[:, gc, 0:1]
        rcol = prr2[:, gc:gc + 1]
        rg = pos.tile([128, 512], BF16, tag="prg5")
        if p1g1 and p1bt0:
            tg = pos.tile([128, 512], F32, tag="ptg5")
            nc.vector.tensor_scalar(
                out=tg, in0=zsb, scalar1=mcol, scalar2=rcol,
                op0=OP.subtract, op1=OP.mult)
            nc.vector.tensor_scalar(
                out=rg, in0=tg, scalar1=0.0, scalar2=None, op0=OP.max)
        else:
            tg = pos.tile([128, 512], F32, tag="ptg5")
            if p1g1:
                nc.vector.tensor_scalar(
                    out=tg, in0=zsb, scalar1=mcol, scalar2=rcol,
                    op0=OP.subtract, op1=OP.mult)
            else:
                nc.vector.scalar_tensor_tensor(
                    out=tg, in0=zsb, scalar=mcol,
                    in1=p1gB, op0=OP.subtract, op1=OP.mult)
                nc.vector.tensor_scalar(
                    out=tg, in0=tg, scalar1=rcol, scalar2=None, op0=OP.mult)
            if not p1bt0:
                nc.vector.tensor_tensor(out=tg, in0=tg, in1=p1btB, op=OP.add)
            nc.vector.tensor_scalar(
                out=rg, in0=tg, scalar1=0.0, scalar2=None, op0=OP.max)
        nc.sync.dma_start_transpose(out=zT[:, :, gsl], in_=rg)

    # final projection
    for gc in range(gchunks):
        gsl = slice(gc * 128, (gc + 1) * 128)
        opp = ps_po.tile([128, 256], F32, tag="cpp", name="opp")
        for kk in range(4):
            nc.tensor.matmul(
                opp, zT[:, kk, gsl], p2w[:, kk, :],
                start=(kk == 0), stop=(p2b0 and kk == 3))
        if not p2b0:
            nc.tensor.matmul(opp, ones1, p2bR, start=False, stop=True)
        osb = pos.tile([128, 256], F32, tag="osb")
        nc.scalar.activation(out=osb, in_=opp, func=AF.Copy)
        nc.sync.dma_start(out=out_d.ap()[gsl, :], in_=osb)


# ---------------------------------------------------------------------------
# host side
# ---------------------------------------------------------------------------

_NC_CACHE = {}


def _get_nc(n_mega=FULL_N_MEGA, flags=None, mm_mode=MM_MODE, repeat=1):
    flags = flags or {}
    key = (n_mega, tuple(sorted(flags.items())), mm_mode, SIM_SAFE, repeat)
    if key not in _NC_CACHE:
        _NC_CACHE[key] = build_nc(n_mega, flags, mm_mode, repeat=repeat)
    return _NC_CACHE[key]


def _bf(a):
    return np.ascontiguousarray(np.asarray(a, np.float32).astype(
        ml_dtypes.bfloat16))


def _f8(a):
    return np.ascontiguousarray(np.asarray(a, np.float32).astype(
        ml_dtypes.float8_e4m3))


def _detect_flags(inp):
    f = {}

    def allz(k):
        return bool((np.asarray(inp[k]) == 0).all())

    def all1(k):
        return bool((np.asarray(inp[k]) == 1).all())

    f["spb0"] = allz("sp_b")
    f["spg1"] = all1("sp_g")
    f["spbt0"] = allz("sp_beta")
    f["b20"] = (allz("att_b2") and allz("piece_b2") and allz("empty_b2"))
    f["cb0"] = allz("comb_b")
    f["cg1"] = all1("comb_g")
    f["cbt0"] = allz("comb_beta")
    f["hb0"] = allz("hier_b")
    f["hg1"] = all1("hier_g")
    f["hbt0"] = allz("hier_beta")
    f["p1b0"] = allz("p1_b")
    f["p1g1"] = all1("p1_g")
    f["p1bt0"] = allz("p1_beta")
    f["p2b0"] = allz("p2_b")
    return f


def _prep_weights(inp, mm_mode=MM_MODE):
    f = np.float32
    att_W1 = np.asarray(inp["att_W1"], f)          # [8, 256, 64]
    att_b1 = np.asarray(inp["att_b1"], f)          # [8, 64]
    att_w2 = np.asarray(inp["att_w2"], f)          # [8, 64]
    piece_W1 = np.asarray(inp["piece_W1"], f)      # [256, 128]
    empty_W1 = np.asarray(inp["empty_W1"], f)
    piece_b1 = np.asarray(inp["piece_b1"], f)      # [128]
    empty_b1 = np.asarray(inp["empty_b1"], f)
    piece_w2 = np.asarray(inp["piece_w2"], f)      # [128]
    empty_w2 = np.asarray(inp["empty_w2"], f)

    w1cat = np.transpose(att_W1, (1, 0, 2)).reshape(256, 512)  # [c, h*64+d]
    w1 = w1cat.reshape(2, 128, 512).transpose(1, 0, 2)         # [p, k, col]
    b1a = np.ascontiguousarray(att_b1.reshape(512).reshape(4, 128).T)
    pecat = np.concatenate([piece_W1, empty_W1], 1)            # [256, 256]
    pew = pecat.reshape(2, 128, 256).transpose(1, 0, 2)
    b1p = np.ascontiguousarray(
        np.concatenate([piece_b1, empty_b1]).reshape(2, 128).T)
    w2c = np.zeros((128, 4, 16), f)
    for h in range(H):
        m, half = divmod(h, 2)
        w2c[64 * half:64 * (half + 1), m, h] = att_w2[h]
    pw2c = np.zeros((128, 2, 16), f)
    pw2c[:, 0, 8] = piece_w2
    pw2c[:, 1, 9] = empty_w2
    b2col = np.zeros((80, 1), f)
    att_b2 = np.asarray(inp["att_b2"], f)
    for h in range(8):
        b2col[h * 8:h * 8 + 8, 0] = att_b2[h]
    b2col[64:72, 0] = np.float32(inp["piece_b2"])
    b2col[72:80, 0] = np.float32(inp["empty_b2"])
    spW = np.asarray(inp["sp_W"], np.float64)
    spW = spW - spW.mean(axis=1, keepdims=True)   # exact LN centering
    spwm = spW.astype(f).reshape(2, 128, 256).transpose(1, 0, 2)
    sav = (1.0 / (1.0 + np.exp(-np.asarray(inp["strat_w"], np.float64))))
    sav = np.tile(sav.reshape(64), 2).astype(f).reshape(128, 1)
    c = np.ascontiguousarray
    wd = {
        "w1": _bf(w1), "b1a": c(b1a), "pew": _bf(pew), "b1p": c(b1p),
        "w2c": _bf(w2c), "pw2c": _bf(pw2c), "b2col": c(b2col),
        "spw": _bf(spwm),
        "spb": _bf((np.asarray(inp["sp_b"], np.float64)
                    - np.asarray(inp["sp_b"], np.float64).mean())
                   .astype(f).reshape(1, 256)),
        "spg": c(np.asarray(inp["sp_g"], f).reshape(1, 256)),
        "spbt": c(np.asarray(inp["sp_beta"], f).reshape(1, 256)),
        "sa": c(sav), "sa2": c(sav * sav),
        "cw": _bf(np.asarray(inp["comb_W"], f).reshape(16, 128, 256)
                  .transpose(1, 0, 2)),
        "cb": _bf(np.asarray(inp["comb_b"], f).reshape(1, 256)),
        "cg": c(np.asarray(inp["comb_g"], f).reshape(1, 256)),
        "cbt": c(np.asarray(inp["comb_beta"], f).reshape(1, 256)),
        "hw": _bf(np.asarray(inp["hier_W"], f).reshape(4, 128, 256)
                  .transpose(1, 0, 2)),
        "hb": _bf(np.asarray(inp["hier_b"], f).reshape(1, 256)),
        "hg": c(np.asarray(inp["hier_g"], f).reshape(1, 256)),
        "hbt": c(np.asarray(inp["hier_beta"], f).reshape(1, 256)),
        "p1w": _bf(np.asarray(inp["p1_W"], f).reshape(6, 128, 512)
                   .transpose(1, 0, 2)),
        "p1b": _bf(np.asarray(inp["p1_b"], f).reshape(1, 512)),
        "p1g": c(np.asarray(inp["p1_g"], f).reshape(1, 512)),
        "p1bt": c(np.asarray(inp["p1_beta"], f).reshape(1, 512)),
        "p2w": _bf(np.asarray(inp["p2_W"], f).reshape(4, 128, 256)
                   .transpose(1, 0, 2)),
        "p2b": _bf(np.asarray(inp["p2_b"], f).reshape(1, 256)),
    }
    if mm_mode == "fp8":
        wd["w1q"] = _f8(w1)
        wd["pewq"] = _f8(pew)
        wd["spwq"] = _f8(spwm)
        # att L2 stationary in DR pairs: [p, i(k of hL pair), mpair, col]
        w2cq = np.zeros((128, 2, 2, 16), f)
        w2cq[:, 0, 0, :] = w2c[:, 0, :]
        w2cq[:, 1, 0, :] = w2c[:, 1, :]
        w2cq[:, 0, 1, :] = w2c[:, 2, :]
        w2cq[:, 1, 1, :] = w2c[:, 3, :]
        wd["w2cq"] = _f8(w2cq)
        wd["pw2cq"] = _f8(pw2c)
    return wd


def make_in_maps(inputs, n_mega=FULL_N_MEGA, mm_mode=MM_MODE):
    x = np.asarray(inputs["x"], np.float32)
    nt = np.asarray(inputs["node_types"])
    mb = (nt > 0).astype(ml_dtypes.bfloat16)
    nb = (nt <= 0).astype(ml_dtypes.bfloat16)
    xb = x.astype(ml_dtypes.bfloat16)
    wd = _prep_weights(inputs, mm_mode)
    nodes_pc = n_mega * MEGA * ST
    in_maps = []
    for cc in range(N_CORES):
        sl = slice(cc * nodes_pc, (cc + 1) * nodes_pc)
        m = {"x": np.ascontiguousarray(xb[sl]),
             "mb": np.ascontiguousarray(mb[sl]),
             "nb": np.ascontiguousarray(nb[sl])}
        m.update(wd)
        in_maps.append(m)
    return in_maps


def run(inputs, n_mega=FULL_N_MEGA, mm_mode=MM_MODE):
    flags = _detect_flags(inputs)
    nc = _get_nc(n_mega, flags, mm_mode)
    in_maps = make_in_maps(inputs, n_mega, mm_mode)
    res = run_bass_kernel_spmd(nc, in_maps, core_ids=list(range(N_CORES)))
    return np.concatenate(
        [res.results[cc]["out"] for cc in range(N_CORES)], axis=0
    )


def kernel(**inputs):
    return run(inputs, FULL_N_MEGA)
